# revision 28
# baseline (speedup 1.0000x reference)
"""Distributed Trainium2 Bass kernel for nn_Attention_25460566131147.

Multi-head attention (B=4, TQ=T=2048, E=2048, H=16, D=128) with gather-based
RoPE and key masking, sharded over 8 NeuronCores: data-parallel over batch
(4 groups) x tensor-parallel over heads (2-way: Wq/Wk/Wv column shards).

Out-projection strategy (v2): instead of each core computing a full-E partial
out-projection and ReduceScattering at the end (leaves ~150us of collectives
on the critical path), each core AllGathers the normalized per-head attention
outputs yt within its pair DURING the attention phase (two AllGathers, fired
after head 3 and head 7), then computes the out-projection for its half of
the E output features with the FULL H*D contraction. The AllGather layout
induces a fixed head-block permutation of the features, which is folded into
a host-side permutation of Wo's rows (rank-independent). The out-projection
reads all 16 head blocks from the gathered DRAM buffers; the contraction is
ordered first-half-first so the second AllGather is hidden behind ~27us of
matmuls on already-arrived data.

Device algorithm (per core, all matmuls bf16 with f32 PSUM accumulation):
  - activations are kept feature-on-partitions (x^T layouts, prepared on host)
  - scores are computed transposed (S^T[k,q] = K_h^T-slice^T @ Q_h^T) so the
    exp'd tile P^T feeds the P@V matmul directly (no on-chip transposes)
  - exp via ScalarE activation with the key-mask folded into the per-partition
    bias and the 1/sqrt(D) scale folded into the activation scale; softmax max-
    subtraction is skipped (scores are O(6), fp32 exp is exact enough)
  - softmax denominator via a binary add-tree of the exp'd tiles on the
    Vector/GpSimd engines plus a single ones-column matmul (instead of TC/2
    ones-matmuls: saves ~47us of TensorE time); reciprocal on VectorE once;
    broadcast back via a float32r ones matmul
  - rotate-half for RoPE via two small SBUF->SBUF DMAs (partition rotation)
  - next-phase weights/tables are prefetched during the previous phase so the
    TensorE never waits on DMA at phase boundaries (also keeps the PE HAM
    clock-gate warm: idle gaps >3.4us re-throttle the array to 1.2GHz)

SBUF pools follow the tile allocator's strict LIFO discipline per side; a
pool's full footprint spans open..close, so prefetch pools are opened in
reverse order of their close times.
"""

import os
import sys

if "JAX_PLATFORMS" in os.environ and os.environ["JAX_PLATFORMS"] == "axon":
    os.environ["JAX_PLATFORMS"] = "axon,cpu"
sys.path.insert(0, "/opt/trn_rl_repo")

import numpy as np
import ml_dtypes

BF16NP = ml_dtypes.bfloat16

B, TQ, T, E, H, D = 4, 2048, 2048, 2048, 16, 128
BLOCK, THETA = 4096, 10000.0
N_CORES = 8
P = 128

# Keys are sorted unmasked-first on the host (softmax is permutation-
# invariant over keys) and truncated to KA: dropped keys are all masked
# (zero attention weight), so the result is exact. Masked keys that remain
# land in the last NMASK key-chunks and are zeroed after exp via a
# per-partition mask multiply — the other chunks need no mask at all, which
# lets exp run as wide bias-free activations (the ACT engine costs
# (N+352)/1.2 ns, so 4-chunk groups cut its fixed cost 4x).
KA = 14 * P               # 1792 active keys kept per batch
NMASK = 2                 # trailing chunks that receive the mask multiply

FULL_CFG = dict(TQ=TQ, T=KA, E=E, HL=8, D=D, NCORES=N_CORES)


def _cs(total, w):
    """Column splits: list of (start, width)."""
    return [(i, min(w, total - i)) for i in range(0, total, w)]


def build_nc(cfg=None):
    """Build and return the (uncompiled) Bacc graph for one SPMD core."""
    import concourse.mybir as mybir
    import concourse.tile as tile
    from concourse import bacc
    from contextlib import ExitStack

    c = dict(FULL_CFG)
    if cfg:
        c.update(cfg)
    cTQ, cT, cE, HL, cD, NCORES = (
        c["TQ"], c["T"], c["E"], c["HL"], c["D"], c["NCORES"],
    )
    assert cD == P
    F = HL * cD              # local feature width (heads shard)
    EC = cE // P             # contraction chunks for projections
    TC = cT // P             # key-position chunks
    NQ = min(512, cTQ)       # q-tile width (PSUM bank limit)
    EH = cE // 2             # out-feature half owned by this core
    BF = mybir.dt.bfloat16
    F32 = mybir.dt.float32
    SCALE = 1.0 / float(np.sqrt(cD))
    groups = [[2 * i, 2 * i + 1] for i in range(NCORES // 2)]

    nc = bacc.Bacc("TRN2", target_bir_lowering=False, debug=False,
                   num_devices=NCORES)

    xt_d = nc.declare_dram_parameter("xt", [cE, cTQ], BF, isOutput=False)
    xat_d = nc.declare_dram_parameter("xat", [cE, cT], BF, isOutput=False)
    wq_d = nc.declare_dram_parameter("wq", [cE, F], BF, isOutput=False)
    wk_d = nc.declare_dram_parameter("wk", [cE, F], BF, isOutput=False)
    wv_d = nc.declare_dram_parameter("wv", [cE, F], BF, isOutput=False)
    # host-permuted Wo rows (AllGather block order), this core's E-col half
    wo_d = nc.declare_dram_parameter("wo", [2 * F, EH], BF, isOutput=False)
    cosq_d = nc.declare_dram_parameter("cosq", [P, cTQ], BF, isOutput=False)
    sinq_d = nc.declare_dram_parameter("sinq", [P, cTQ], BF, isOutput=False)
    cosk_d = nc.declare_dram_parameter("cosk", [P, cT], BF, isOutput=False)
    sink_d = nc.declare_dram_parameter("sink", [P, cT], BF, isOutput=False)
    mb_d = nc.declare_dram_parameter("mask01", [P, TC], F32, isOutput=False)
    out_d = nc.declare_dram_parameter("out", [EH, cTQ], BF, isOutput=True)

    # yt exchange buffers: own 2-head blocks, gathered pair blocks
    HH = 2                   # heads per AllGather
    NAG = HL // HH           # number of AllGathers (4)
    agin = [nc.dram_tensor(f"agin{h}", [HH * P, cTQ], BF) for h in range(NAG)]
    agout = [nc.dram_tensor(f"agout{h}", [2 * HH * P, cTQ], BF)
             for h in range(NAG)]

    with tile.TileContext(nc) as tc, ExitStack() as ex:
        # right side: persistent accumulating tiles; left side: phase-scoped
        consts = ex.enter_context(tc.tile_pool(name="consts", bufs=1, side="right"))
        ones_bf = consts.tile([P, 1], BF, tag="ones_bf", name="ones_bf")
        nc.vector.memset(ones_bf[:], 1.0)
        mb_sb = consts.tile([P, TC], F32, tag="mask01", name="mask01")
        nc.sync.dma_start(mb_sb[:], mb_d[:])
        # packed denominators: head m lives at partition base (m%4)*32
        # (engine ops need 32-aligned start partitions), column (m//4)*128
        den_sb = consts.tile([P, 2 * P], F32, tag="den", name="den")
        ones_fr = consts.tile([1, P], F32, tag="ones_fr", name="ones_fr")
        nc.vector.memset(ones_fr[:], 1.0)

        vp = ex.enter_context(tc.tile_pool(name="v", bufs=1, side="right"))
        es_kqv = ExitStack()  # kt/qt pools: closed before phase D (SBUF reuse)

        # left-side pool stack, opened in reverse close order (LIFO):
        es_proj = ExitStack()   # projection psum: [V .. Q]
        warmp = es_proj.enter_context(
            tc.tile_pool(name="warm", bufs=1, space="PSUM"))
        psproj = es_proj.enter_context(
            tc.tile_pool(name="psproj", bufs=2, space="PSUM"))
        es_tabq = ExitStack()   # Q-phase weights+tables: [pre-V .. Q]
        wqp = es_tabq.enter_context(tc.tile_pool(name="wq", bufs=1))
        tabq = es_tabq.enter_context(tc.tile_pool(name="tabq", bufs=1))
        es_xt = ExitStack()     # x^T quarters: [pre-V .. Q]
        xtp = es_xt.enter_context(tc.tile_pool(name="xt", bufs=1))
        es_tabk = ExitStack()   # K-phase weights+tables: [pre-V .. K]
        wkp = es_tabk.enter_context(tc.tile_pool(name="wk", bufs=1))
        tabk = es_tabk.enter_context(tc.tile_pool(name="tabk", bufs=1))
        es_xak = ExitStack()    # xall^T quarters for K: [pre-V .. K]
        xakp = es_xak.enter_context(tc.tile_pool(name="xak", bufs=1))

        SEG = min(512, cT)   # projection/rope column-segment width

        def proj_rope(m, w_sb, src_tiles, src_c0, out_c0, width, cos_sb,
                      sin_sb, out_t, tg, rawp, tmpp):
            """Project head m (cols [src_c0, src_c0+width) of src) and apply
            RoPE, writing cols [out_c0, out_c0+width) of out_t."""
            raw = rawp.tile([P, width], BF, tag=f"raw{tg}", name=f"raw{tg}")
            swp = rawp.tile([P, width], BF, tag=f"swp{tg}", name=f"swp{tg}")
            ps = psproj.tile([P, SEG], F32, tag="projps", name="projps")
            for e in range(EC):
                for ns, nw in _cs(width, 512):
                    nc.tensor.matmul(
                        ps[:, ns:ns + nw],
                        w_sb[e][:, m * P:(m + 1) * P],
                        src_tiles[e][:, src_c0 + ns:src_c0 + ns + nw],
                        start=(e == 0), stop=(e == EC - 1),
                    )
            nc.scalar.copy(raw[:], ps[:, 0:width])
            # partition rotate-half via SBUF->SBUF DMA (cross-partition)
            half = P // 2
            nc.sync.dma_start(swp[0:half, :], raw[half:P, :])
            nc.sync.dma_start(swp[half:P, :], raw[0:half, :])
            t1 = tmpp.tile([P, width], BF, tag="rope_t1", name="rope_t1")
            t2 = tmpp.tile([P, width], BF, tag="rope_t2", name="rope_t2")
            nc.vector.tensor_mul(t1[:], raw[:], cos_sb[:, out_c0:out_c0 + width])
            nc.vector.tensor_mul(t2[:], swp[:], sin_sb[:, out_c0:out_c0 + width])
            nc.vector.tensor_add(out_t[:, out_c0:out_c0 + width], t1[:], t2[:])

        # ============ phase V: V = xall @ Wv, [t-part, n-free] ===========
        # xall^T is streamed in 512-column chunks (and re-streamed for K)
        # to bound SBUF.
        assert F <= 1024
        v_sb = [vp.tile([P, F], BF, tag=f"v{t}", name=f"v{t}")
                for t in range(TC)]
        with tc.tile_pool(name="wv", bufs=1) as wvp, \
                tc.tile_pool(name="xav", bufs=1) as xavp:
            wv_sb = []
            for e in range(EC):
                t_ = wvp.tile([P, F], BF, tag=f"wv{e}", name=f"wv{e}")
                wv_sb.append(t_)
            # first compute tile needs all xav e-chunks of seg 0 plus wv[0];
            # emit those DMAs first, then the rest of wv
            seg0_xa = []
            for e in range(EC):
                t_ = xavp.tile([P, SEG], BF, tag=f"xav{e}", name=f"xav{e}")
                nc.sync.dma_start(t_[:], xat_d[e * P:(e + 1) * P, 0:SEG])
                seg0_xa.append(t_)
            nc.sync.dma_start(wv_sb[0][:], wv_d[0:P, :])
            for e in range(1, EC):
                nc.sync.dma_start(wv_sb[e][:], wv_d[e * P:(e + 1) * P, :])
            # PE warm-up chain: ~50 cheap matmuls with no input deps keep the
            # HAM activity monitor busy during the initial DMA wait so the
            # first real matmuls run at 2.4GHz instead of 1.2GHz
            dumw = wvp.tile([P, 512], BF, tag="dumw", name="dumw")
            nc.vector.memset(dumw[:], 0.0)
            wps = warmp.tile([1, 512], F32, tag="wps", name="wps")
            for _ in range(48):
                nc.tensor.matmul(wps[0:1, :], ones_bf[:, 0:1], dumw[:],
                                 start=True, stop=True)
            # prefetch K-phase tables+weights (used next phase)
            cosk_sb = tabk.tile([P, cT], BF, tag="cosk", name="cosk")
            sink_sb = tabk.tile([P, cT], BF, tag="sink", name="sink")
            nc.sync.dma_start(cosk_sb[:], cosk_d[:])
            nc.sync.dma_start(sink_sb[:], sink_d[:])
            wk_sb = []
            for e in range(EC):
                t_ = wkp.tile([P, F], BF, tag=f"wk{e}", name=f"wk{e}")
                nc.sync.dma_start(t_[:], wk_d[e * P:(e + 1) * P, :])
                wk_sb.append(t_)
            for h0, hw in _cs(cT, SEG):
                if h0 == 0:
                    xa_sb = seg0_xa
                else:
                    xa_sb = []
                    for e in range(EC):
                        t_ = xavp.tile([P, SEG], BF, tag=f"xav{e}", name=f"xav{e}")
                        nc.sync.dma_start(
                            t_[:, 0:hw], xat_d[e * P:(e + 1) * P, h0:h0 + hw])
                        xa_sb.append(t_)
                for tl in range(hw // P):
                    t = (h0 // P) + tl
                    ps = psproj.tile([P, F], F32, tag="projpsv", name="projpsv")
                    for e in range(EC):
                        for ns, nw in _cs(F, 512):
                            nc.tensor.matmul(
                                ps[:, ns:ns + nw],
                                xa_sb[e][:, tl * P:(tl + 1) * P],
                                wv_sb[e][:, ns:ns + nw],
                                start=(e == 0), stop=(e == EC - 1),
                            )
                    nc.vector.tensor_copy(v_sb[t][:], ps[:, 0:F])

        # ============ phase K: K-proj + RoPE =============================
        ktp = es_kqv.enter_context(tc.tile_pool(name="kt", bufs=1, side="right"))
        kt_sb = [ktp.tile([P, cT], BF, tag=f"kt{m}", name=f"kt{m}")
                 for m in range(HL)]
        # 448-wide segments divide the trimmed key length evenly (a 256-wide
        # tail segment would expose LDWEIGHTS behind short streams)
        SEGK = 448 if cT % 448 == 0 else SEG
        with tc.tile_pool(name="rawk", bufs=1) as rawkp, \
                tc.tile_pool(name="tmpk", bufs=2) as tmpkp:
            first = True
            for h0, hw in _cs(cT, SEGK):
                xa_sb = []
                for e in range(EC):
                    t_ = xakp.tile([P, SEG], BF, tag=f"xak{e}", name=f"xak{e}")
                    nc.sync.dma_start(
                        t_[:, 0:hw], xat_d[e * P:(e + 1) * P, h0:h0 + hw])
                    xa_sb.append(t_)
                if first:
                    # prefetch Q-phase tables+weights behind seg-0 loads
                    first = False
                    cosq_sb = tabq.tile([P, cTQ], BF, tag="cosq", name="cosq")
                    sinq_sb = tabq.tile([P, cTQ], BF, tag="sinq", name="sinq")
                    nc.sync.dma_start(cosq_sb[:], cosq_d[:])
                    nc.sync.dma_start(sinq_sb[:], sinq_d[:])
                    wq_sb = []
                    for e in range(EC):
                        t_ = wqp.tile([P, F], BF, tag=f"wq{e}", name=f"wq{e}")
                        nc.sync.dma_start(t_[:], wq_d[e * P:(e + 1) * P, :])
                        wq_sb.append(t_)
                for m in range(HL):
                    proj_rope(m, wk_sb, xa_sb, 0, h0, hw, cosk_sb,
                              sink_sb, kt_sb[m], "k", rawkp, tmpkp)
        es_xak.close()
        es_tabk.close()

        # ============ phase Q: Q-proj + RoPE (x^T in quarters) ===========
        qtp = es_kqv.enter_context(tc.tile_pool(name="qt", bufs=1, side="right"))
        qt_sb = []
        for m in range(HL):
            qt_sb.append(qtp.tile([P, cTQ], BF, tag=f"qt{m}", name=f"qt{m}"))
        with tc.tile_pool(name="rawq", bufs=1) as rawqp, \
                tc.tile_pool(name="tmpq", bufs=2) as tmpqp:
            TQH = min(512, cTQ)
            for th, (h0, hw) in enumerate(_cs(cTQ, TQH)):
                xt_sb = []
                for e in range(EC):
                    t_ = xtp.tile([P, TQH], BF, tag=f"xt{e}", name=f"xt{e}")
                    nc.sync.dma_start(
                        t_[:], xt_d[e * P:(e + 1) * P, h0:h0 + hw])
                    xt_sb.append(t_)
                for m in range(HL):
                    proj_rope(m, wq_sb, xt_sb, 0, h0, hw, cosq_sb, sinq_sb,
                              qt_sb[m], "q", rawqp, tmpqp)
        es_xt.close()
        es_tabq.close()
        es_proj.close()

        # ================= phase C: attention ============================
        FR = mybir.dt.float32r
        RPM = cTQ // P                # packed den rows per head
        es_wo = ExitStack()     # out-proj weights, loaded during attention
        wop = es_wo.enter_context(tc.tile_pool(name="wo", bufs=1))
        es_ya = ExitStack()     # first 4 gathered yt tiles (loaded in C)
        ya1p = es_ya.enter_context(tc.tile_pool(name="ya1", bufs=1))
        es_att = ExitStack()
        ptp = es_att.enter_context(tc.tile_pool(name="pt", bufs=2))
        pt2p = es_att.enter_context(tc.tile_pool(name="pt2", bufs=4))
        accp = es_att.enter_context(tc.tile_pool(name="acc", bufs=2))
        ytp = es_att.enter_context(tc.tile_pool(name="yt", bufs=2))
        dstp = es_att.enter_context(tc.tile_pool(name="dst", bufs=2))
        dnerp = es_att.enter_context(tc.tile_pool(name="dner", bufs=1))
        pss = es_att.enter_context(tc.tile_pool(name="pss", bufs=2, space="PSUM"))
        psy = es_att.enter_context(tc.tile_pool(name="psy", bufs=2, space="PSUM"))
        psb = es_att.enter_context(tc.tile_pool(name="psb", bufs=2, space="PSUM"))

        wo_sb = [None] * (2 * F // P)
        ya_sb = []
        GW = 2                    # score chunks per exp group (PSUM-limited)
        GR = [(i, min(i + GW, TC)) for i in range(0, TC, GW)]

        # The denominator reduction for q-block (m, qs) is "flushed" (its
        # ones-matmul + psum evacuations) early in the NEXT q-block, so the
        # in-order TensorE queue never waits on the VectorE add chain.
        def flush_den(pend):
            fm, fqs, fqw, facc, fyps, fyt = pend
            dps = psb.tile([P, NQ], F32, tag="dbc", name="dps")
            nc.tensor.matmul(
                dps[0:1, 0:fqw],
                ones_bf[:, 0:1],
                facc[:, 0:fqw],
                start=True, stop=True,
            )
            nc.vector.tensor_copy(fyt[:, fqs:fqs + fqw], fyps[:, 0:fqw])
            dst = dstp.tile([1, NQ], F32, tag="dst", name="dst")
            nc.vector.tensor_copy(dst[0:1, 0:fqw], dps[0:1, 0:fqw])
            # scatter the denominator row into the packed layout
            # (DMA can cross partitions)
            bp = (fm % 4) * 32 + fqs // P
            c0 = (fm // 4) * P
            nc.sync.dma_start(
                den_sb[bp:bp + fqw // P, c0:c0 + P], dst[0:1, 0:fqw])

        def normalize_ship(fm, fyt):
            # head fm normalization (runs while head fm+1 attention computes)
            bp = (fm % 4) * 32
            c0 = (fm // 4) * P
            nc.vector.reciprocal(den_sb[bp:bp + RPM, c0:c0 + P],
                                 den_sb[bp:bp + RPM, c0:c0 + P])
            dner = dnerp.tile([1, cTQ], F32, tag="dner", name="dner")
            nc.sync.dma_start(dner[0:1, :],
                              den_sb[bp:bp + RPM, c0:c0 + P])
            for qs, qw in _cs(cTQ, NQ):
                dbc = psb.tile([P, NQ], F32, tag="dbc", name="dbc")
                nc.tensor.matmul(
                    dbc[:, 0:qw],
                    ones_fr[0:1, :].bitcast(FR),
                    dner[0:1, qs:qs + qw].bitcast(FR),
                    start=True, stop=True,
                )
                nc.vector.tensor_mul(
                    fyt[:, qs:qs + qw],
                    fyt[:, qs:qs + qw],
                    dbc[:, 0:qw],
                )
            # ship normalized head to the pair-exchange buffer; fire the
            # block's AllGather once both of its heads have landed
            blk, ml = divmod(fm, HH)
            nc.sync.dma_start(agin[blk][ml * P:(ml + 1) * P, :], fyt[:])
            if ml == HH - 1:
                nc.gpsimd.collective_compute(
                    "AllGather",
                    mybir.AluOpType.bypass,
                    replica_groups=groups,
                    ins=[agin[blk][:]],
                    outs=[agout[blk][:]],
                )
            if fm == 3:
                # first gathered block (4 tiles): DMA in during heads 4-7
                for f in range(2 * HH):
                    t_ = ya1p.tile([P, cTQ], BF, tag=f"ya{f}", name=f"ya{f}")
                    nc.sync.dma_start(t_[:], agout[0][f * P:(f + 1) * P, :])
                    ya_sb.append(t_)

        assert cTQ % NQ == 0

        def finish_group(g, st, m):
            """Exp + mask + P@V + denominator adds for score group g."""
            c0, c1 = GR[g]
            w = (c1 - c0) * NQ
            sps = st["sps"][g]
            pt = ptp.tile([P, GW * NQ], BF, tag="pt", name="pt")
            nc.scalar.activation(
                pt[:, 0:w], sps[:, 0:w],
                mybir.ActivationFunctionType.Exp, scale=SCALE,
            )
            for kc in range(max(c0, TC - NMASK), c1):
                j = kc - c0
                nc.vector.tensor_scalar_mul(
                    pt[:, j * NQ:(j + 1) * NQ],
                    pt[:, j * NQ:(j + 1) * NQ],
                    mb_sb[:, kc:kc + 1],
                )
            for kc in range(c0, c1):
                j = kc - c0
                nc.tensor.matmul(
                    st["yps"][:, 0:NQ],
                    v_sb[kc][:, m * P:(m + 1) * P],
                    pt[:, j * NQ:(j + 1) * NQ],
                    start=(kc == 0), stop=(kc == TC - 1),
                )
            for j0 in range(0, c1 - c0, 2):
                pt2 = pt2p.tile([P, NQ], BF, tag="pt2", name="pt2")
                nc.vector.tensor_add(pt2[:], pt[:, j0 * NQ:(j0 + 1) * NQ],
                                     pt[:, (j0 + 1) * NQ:(j0 + 2) * NQ])
                if st["acc"] is None:
                    st["acc"] = pt2
                else:
                    nacc = accp.tile([P, NQ], BF, tag="acc", name="acc")
                    nc.vector.tensor_add(nacc[:], st["acc"][:], pt2[:])
                    st["acc"] = nacc

        pending = None            # last q-block awaiting its denominator flush
        pending_head = None       # last head awaiting normalization
        for m in range(HL):
            yt_t = ytp.tile([P, cTQ], BF, tag="yt", name=f"yt{m}")
            if 1 <= m <= 4:
                # out-proj weights, 4 tiles per head: spread so these bulk
                # DMAs never sit ahead of the small denominator DMAs in the
                # queue (a 4MB burst at phase start stalled VectorE 10us)
                for f in range(4 * (m - 1), 4 * m):
                    t_ = wop.tile([P, EH], BF, tag=f"wo{f}", name=f"wo{f}")
                    nc.sync.dma_start(t_[:], wo_d[f * P:(f + 1) * P, :])
                    wo_sb[f] = t_
            for qs, qw in _cs(cTQ, NQ):
                st = {
                    "yps": psy.tile([P, NQ], F32, tag="yps", name="yps"),
                    "sps": [],
                    "acc": None,
                }
                for g, (c0, c1) in enumerate(GR):
                    sps = pss.tile([P, GW * NQ], F32, tag="sps", name="sps")
                    st["sps"].append(sps)
                    for kc in range(c0, c1):
                        j = kc - c0
                        nc.tensor.matmul(
                            sps[:, j * NQ:j * NQ + qw],
                            kt_sb[m][:, kc * P:(kc + 1) * P],
                            qt_sb[m][:, qs:qs + qw],
                            start=True, stop=True,
                        )
                    if g >= 1:
                        finish_group(g - 1, st, m)
                    if g == 1:
                        # previous block's denominator + evacuations go on
                        # the queues here, behind this block's first matmuls
                        if pending is not None:
                            flush_den(pending)
                            pending = None
                        if pending_head is not None:
                            normalize_ship(*pending_head)
                            pending_head = None
                finish_group(len(GR) - 1, st, m)
                pending = (m, qs, qw, st["acc"], st["yps"], yt_t)
            pending_head = (m, yt_t)
        flush_den(pending)
        normalize_ship(*pending_head)
        es_att.close()
        es_kqv.close()

        # ================= phase D: out-projection =======================
        # out^T[EH, q] = Wo'^T @ ya. The f contraction for each ms block is
        # emitted in three sweeps over all 8 PSUM banks — f 0..3 (in SBUF
        # since mid-attention), f 4..11 (gathered blocks 1-2, DMA'd at D
        # start), f 12..15 (gathered block 3) — so ~20us of matmuls on
        # already-arrived data hide the final AllGather and its loads.
        NT = EH // P
        NF = 2 * HL
        with tc.tile_pool(name="ya2", bufs=1) as ya2p, \
                tc.tile_pool(name="oev", bufs=4) as oevp, \
                tc.tile_pool(name="pso", bufs=8, space="PSUM") as pso:
            for f in range(2 * HH, NF):
                blk, r = divmod(f, 2 * HH)
                t_ = ya2p.tile([P, cTQ], BF, tag=f"yb{f}", name=f"yb{f}")
                nc.sync.dma_start(t_[:], agout[blk][r * P:(r + 1) * P, :])
                ya_sb.append(t_)
            sweeps = [(0, 4), (4, 12), (12, NF)]
            for ms, mw in _cs(cTQ, 512):
                opss = []
                for si, (f0, f1) in enumerate(sweeps):
                    for n in range(NT):
                        if si == 0:
                            ops = pso.tile([P, 512], F32, tag="ops", name="ops")
                            opss.append(ops)
                        else:
                            ops = opss[n]
                        for f in range(f0, f1):
                            nc.tensor.matmul(
                                ops[:, 0:mw],
                                wo_sb[f][:, n * P:(n + 1) * P],
                                ya_sb[f][:, ms:ms + mw],
                                start=(f == 0), stop=(f == NF - 1),
                            )
                        if si == len(sweeps) - 1:
                            oev = oevp.tile([P, 512], BF, tag="oev", name="oev")
                            if n % 2 == 0:
                                nc.scalar.copy(oev[:, 0:mw], ops[:, 0:mw])
                            else:
                                nc.vector.tensor_copy(oev[:, 0:mw],
                                                      ops[:, 0:mw])
                            nc.sync.dma_start(
                                out_d[n * P:(n + 1) * P, ms:ms + mw],
                                oev[:, 0:mw])
        es_ya.close()
        es_wo.close()

    return nc


# ---------------------------------------------------------------------------
# host side
# ---------------------------------------------------------------------------

def _rope_tables():
    inv_freq = 1.0 / (THETA ** (np.arange(0, D, 2, dtype=np.float32) / D))
    t = np.arange(BLOCK, dtype=np.float32)
    freqs = np.einsum("i,j->ij", t, inv_freq).astype(np.float32)
    emb = np.concatenate([freqs, freqs], axis=-1)
    return np.cos(emb).astype(np.float32), np.sin(emb).astype(np.float32)


_NC_CACHE = {}


def _get_compiled():
    if "nc" not in _NC_CACHE:
        nc = build_nc()
        nc.compile()
        _NC_CACHE["nc"] = nc
    return _NC_CACHE["nc"]


def _bf(a):
    return np.ascontiguousarray(a).astype(BF16NP)


def prepare_in_maps(x, xall, posx, posxall, mask, Wq, Wk, Wv, Wo):
    x = np.asarray(x, dtype=np.float32)
    xall = np.asarray(xall, dtype=np.float32)
    posx = np.asarray(posx)
    posxall = np.asarray(posxall)
    mask = np.asarray(mask)
    Wq = np.asarray(Wq, dtype=np.float32)
    Wk = np.asarray(Wk, dtype=np.float32)
    Wv = np.asarray(Wv, dtype=np.float32)
    Wo = np.asarray(Wo, dtype=np.float32)

    cos_t, sin_t = _rope_tables()
    sign = np.ones((1, D), np.float32)
    sign[0, : D // 2] = -1.0

    F = (H * D) // 2  # 1024: per-core head-shard width
    FB = 2 * D        # 256: AllGather block (2 heads)
    # AllGather block order: [A blk_i, B blk_i] for i in 0..3, where A/B are
    # the pair's rank-0/rank-1 feature halves of Wo's rows
    Wo_perm = np.concatenate(
        [w for i in range(4)
         for w in (Wo[i * FB:(i + 1) * FB], Wo[F + i * FB:F + (i + 1) * FB])],
        axis=0)

    in_maps = []
    for c in range(N_CORES):
        b, hg = c // 2, c % 2
        sl = slice(hg * F, (hg + 1) * F)
        # sort keys unmasked-first (stable) and keep the first KA: every
        # dropped key is masked (zero attention weight) so this is exact;
        # remaining masked keys land in the last NMASK chunks
        order = np.argsort(mask[b], kind="stable")[:KA]
        act = int((~mask[b]).sum())
        assert act <= KA, f"batch {b}: {act} active keys > {KA}"
        assert KA - act <= NMASK * P, \
            f"batch {b}: masked keys spill out of the last {NMASK} chunks"
        xall_p = xall[b][order]
        posk_p = posxall[b][order]
        mask_p = mask[b][order]
        cosq = _bf(cos_t[posx[b]].T)                    # [128, TQ]
        sinq = _bf((sin_t[posx[b]] * sign).T)
        cosk = _bf(cos_t[posk_p].T)                     # [128, KA]
        sink = _bf((sin_t[posk_p] * sign).T)
        m01 = np.where(mask_p, np.float32(0.0), np.float32(1.0))
        m01 = np.ascontiguousarray(m01.reshape(KA // P, P).T)  # [128, TCA]
        in_maps.append({
            "xt": _bf(x[b].T),
            "xat": _bf(xall_p.T),
            "wq": _bf(Wq[:, sl]),
            "wk": _bf(Wk[:, sl]),
            "wv": _bf(Wv[:, sl]),
            "wo": _bf(Wo_perm[:, hg * (E // 2):(hg + 1) * (E // 2)]),
            "cosq": cosq, "sinq": sinq, "cosk": cosk, "sink": sink,
            "mask01": m01.astype(np.float32),
        })
    return in_maps


def assemble_out(results):
    # core (b, hg) computed out^T for E columns [hg*E/2, (hg+1)*E/2)
    EH = E // 2
    out = np.empty((B, TQ, E), np.float32)
    for b in range(B):
        for hg in range(2):
            half = results[2 * b + hg]["out"].astype(np.float32)
            out[b][:, hg * EH:(hg + 1) * EH] = half.T
    return out


def kernel(x, xall, posx, posxall, mask, Wq, Wk, Wv, Wo):
    from concourse.bass_utils import run_bass_kernel_spmd

    in_maps = prepare_in_maps(x, xall, posx, posxall, mask, Wq, Wk, Wv, Wo)
    nc = _get_compiled()
    res = run_bass_kernel_spmd(nc, in_maps, list(range(N_CORES)), trace=False)
    return assemble_out(res.results)


# revision 30
# speedup vs baseline: 1.1013x; 1.1013x over previous
"""Distributed Trainium2 Bass kernel for nn_Attention_25460566131147.

Multi-head attention (B=4, TQ=T=2048, E=2048, H=16, D=128) with gather-based
RoPE and key masking, sharded over 8 NeuronCores: data-parallel over batch
(4 groups) x tensor-parallel over heads (2-way: Wq/Wk/Wv column shards).

Structure (v4):
  - keys are sorted unmasked-first on the host (softmax is permutation-
    invariant over keys) and truncated to KA=1792: dropped keys are all
    masked (zero attention weight) so the result is exact. Remaining masked
    keys land in the last NMASK chunks and are zeroed after exp via a
    per-partition mask multiply; every other chunk needs no mask, letting
    exp run as wide bias-free activations (ACT costs (N+352)/1.2 ns).
  - the Q projection is FUSED into the attention phase: attention runs
    q-block-outer / head-inner, and each sweep first projects+RoPEs its own
    512-wide q-block for all heads. Attention is ScalarE(exp)-bound while
    projections are TensorE-bound, so fusing fills each engine's bubbles.
  - softmax normalization is per (head, q-block) via a two-stage pipelined
    flush (denominator ones-matmul + reciprocal one block later, broadcast
    multiply + ship one block after that) so the in-order TensorE queue
    never waits on the VectorE chain.
  - normalized yt q-slices are AllGathered within the pair in 8 pieces
    (head-pair x q-half) as they complete; the out-projection contracts the
    gathered blocks with a host-permuted Wo (rank-independent) computing
    this core's E-half, ordered so the final AllGathers are hidden behind
    matmuls on already-arrived data.
  - scores are computed transposed (S^T[k,q]) so the exp'd tile feeds P@V
    directly; the softmax denominator comes from a pairwise add + running
    accumulator on VectorE plus a single ones-column matmul (GpSimd is
    avoided: each collective trigger blocks its queue ~20us).
  - a PE warm-up matmul chain covers the initial DMA wait so the HAM clock
    gate reaches 2.4GHz before real work; phase weights/tables are
    prefetched a phase ahead (SBUF pools are strict LIFO per side).
"""

import os
import sys

if "JAX_PLATFORMS" in os.environ and os.environ["JAX_PLATFORMS"] == "axon":
    os.environ["JAX_PLATFORMS"] = "axon,cpu"
sys.path.insert(0, "/opt/trn_rl_repo")

import numpy as np
import ml_dtypes

BF16NP = ml_dtypes.bfloat16

B, TQ, T, E, H, D = 4, 2048, 2048, 2048, 16, 128
BLOCK, THETA = 4096, 10000.0
N_CORES = 8
P = 128

KA = 14 * P               # 1792 active keys kept per batch
NMASK = 2                 # trailing chunks that receive the mask multiply

FULL_CFG = dict(TQ=TQ, T=KA, E=E, HL=8, D=D, NCORES=N_CORES)


def _cs(total, w):
    """Column splits: list of (start, width)."""
    return [(i, min(w, total - i)) for i in range(0, total, w)]


def build_nc(cfg=None):
    """Build and return the (uncompiled) Bacc graph for one SPMD core."""
    import concourse.mybir as mybir
    import concourse.tile as tile
    from concourse import bacc
    from contextlib import ExitStack

    c = dict(FULL_CFG)
    if cfg:
        c.update(cfg)
    cTQ, cT, cE, HL, cD, NCORES = (
        c["TQ"], c["T"], c["E"], c["HL"], c["D"], c["NCORES"],
    )
    assert cD == P
    F = HL * cD              # local feature width (heads shard)
    EC = cE // P             # contraction chunks for projections
    TC = cT // P             # active key chunks (14)
    NQ = 512                 # q-block width
    HQ = cTQ // 2            # q-half width (AllGather granule)
    EH = cE // 2             # out-feature half owned by this core
    BF = mybir.dt.bfloat16
    F32 = mybir.dt.float32
    FR = mybir.dt.float32r
    SCALE = 1.0 / float(np.sqrt(cD))
    groups = [[2 * i, 2 * i + 1] for i in range(NCORES // 2)]

    nc = bacc.Bacc("TRN2", target_bir_lowering=False, debug=False,
                   num_devices=NCORES)

    xt_d = nc.declare_dram_parameter("xt", [cE, cTQ], BF, isOutput=False)
    xat_d = nc.declare_dram_parameter("xat", [cE, cT], BF, isOutput=False)
    wq_d = nc.declare_dram_parameter("wq", [cE, F], BF, isOutput=False)
    wk_d = nc.declare_dram_parameter("wk", [cE, F], BF, isOutput=False)
    wv_d = nc.declare_dram_parameter("wv", [cE, F], BF, isOutput=False)
    # host-permuted Wo rows (AllGather block order), this core's E-col half
    wo_d = nc.declare_dram_parameter("wo", [2 * F, EH], BF, isOutput=False)
    cosq_d = nc.declare_dram_parameter("cosq", [P, cTQ], BF, isOutput=False)
    sinq_d = nc.declare_dram_parameter("sinq", [P, cTQ], BF, isOutput=False)
    cosk_d = nc.declare_dram_parameter("cosk", [P, cT], BF, isOutput=False)
    sink_d = nc.declare_dram_parameter("sink", [P, cT], BF, isOutput=False)
    mb_d = nc.declare_dram_parameter("mask01", [P, TC], F32, isOutput=False)
    out_d = nc.declare_dram_parameter("out", [EH, cTQ], BF, isOutput=True)

    # yt exchange: 8 buffers = (head-pair block) x (q half)
    NBLK = HL // 2
    agin = [nc.dram_tensor(f"agin{j}", [2 * P, HQ], BF) for j in range(8)]
    agout = [nc.dram_tensor(f"agout{j}", [4 * P, HQ], BF) for j in range(8)]

    with tile.TileContext(nc) as tc, ExitStack() as ex:
        consts = ex.enter_context(tc.tile_pool(name="consts", bufs=1, side="right"))
        ones_bf = consts.tile([P, 1], BF, tag="ones_bf", name="ones_bf")
        nc.vector.memset(ones_bf[:], 1.0)
        mb_sb = consts.tile([P, TC], F32, tag="mask01", name="mask01")
        nc.sync.dma_start(mb_sb[:], mb_d[:])
        ones_fr = consts.tile([1, P], F32, tag="ones_fr", name="ones_fr")
        nc.vector.memset(ones_fr[:], 1.0)

        vp = ex.enter_context(tc.tile_pool(name="v", bufs=1, side="right"))
        es_kqv = ExitStack()  # kt pool: closed before phase D (SBUF reuse)

        # left-side pool stack, opened in reverse close order (LIFO):
        es_proj = ExitStack()   # V/K projection psum: [V .. K]
        warmp = es_proj.enter_context(
            tc.tile_pool(name="warm", bufs=1, space="PSUM"))
        psproj = es_proj.enter_context(
            tc.tile_pool(name="psproj", bufs=2, space="PSUM"))
        es_tabq = ExitStack()   # Q weights+tables: [pre-V .. end]
        wqp = es_tabq.enter_context(tc.tile_pool(name="wq", bufs=1))
        tabq = es_tabq.enter_context(tc.tile_pool(name="tabq", bufs=1))
        es_xt = ExitStack()     # x^T quarters: [pre-V .. end]
        xtp = es_xt.enter_context(tc.tile_pool(name="xt", bufs=1))
        es_tabk = ExitStack()   # K weights+tables: [pre-V .. K]
        wkp = es_tabk.enter_context(tc.tile_pool(name="wk", bufs=1))
        tabk = es_tabk.enter_context(tc.tile_pool(name="tabk", bufs=1))
        es_xak = ExitStack()    # xall^T quarters for K: [pre-V .. K]
        xakp = es_xak.enter_context(tc.tile_pool(name="xak", bufs=1))

        SEG = min(512, cT)   # projection column-segment width

        # ============ phase V: V = xall @ Wv, [t-part, n-free] ===========
        assert F <= 1024
        v_sb = [vp.tile([P, F], BF, tag=f"v{t}", name=f"v{t}")
                for t in range(TC)]
        with tc.tile_pool(name="wv", bufs=1) as wvp, \
                tc.tile_pool(name="xav", bufs=1) as xavp:
            wv_sb = []
            for e in range(EC):
                t_ = wvp.tile([P, F], BF, tag=f"wv{e}", name=f"wv{e}")
                wv_sb.append(t_)
            # critical first tiles first, then the rest
            seg0_xa = []
            for e in range(EC):
                t_ = xavp.tile([P, SEG], BF, tag=f"xav{e}", name=f"xav{e}")
                nc.sync.dma_start(t_[:], xat_d[e * P:(e + 1) * P, 0:SEG])
                seg0_xa.append(t_)
            nc.sync.dma_start(wv_sb[0][:], wv_d[0:P, :])
            for e in range(1, EC):
                nc.sync.dma_start(wv_sb[e][:], wv_d[e * P:(e + 1) * P, :])
            # PE warm-up chain: matmuls with no input deps keep the HAM
            # activity monitor busy during the initial DMA wait so the
            # first real matmuls run at 2.4GHz instead of 1.2GHz
            dumw = wvp.tile([P, 512], BF, tag="dumw", name="dumw")
            nc.vector.memset(dumw[:], 0.0)
            wps = warmp.tile([1, 512], F32, tag="wps", name="wps")
            for _ in range(48):
                nc.tensor.matmul(wps[0:1, :], ones_bf[:, 0:1], dumw[:],
                                 start=True, stop=True)
            # prefetch K-phase tables+weights (used next phase)
            cosk_sb = tabk.tile([P, cT], BF, tag="cosk", name="cosk")
            sink_sb = tabk.tile([P, cT], BF, tag="sink", name="sink")
            nc.sync.dma_start(cosk_sb[:], cosk_d[:])
            nc.sync.dma_start(sink_sb[:], sink_d[:])
            wk_sb = []
            for e in range(EC):
                t_ = wkp.tile([P, F], BF, tag=f"wk{e}", name=f"wk{e}")
                nc.sync.dma_start(t_[:], wk_d[e * P:(e + 1) * P, :])
                wk_sb.append(t_)
            for h0, hw in _cs(cT, SEG):
                if h0 == 0:
                    xa_sb = seg0_xa
                else:
                    xa_sb = []
                    for e in range(EC):
                        t_ = xavp.tile([P, SEG], BF, tag=f"xav{e}", name=f"xav{e}")
                        nc.sync.dma_start(
                            t_[:, 0:hw], xat_d[e * P:(e + 1) * P, h0:h0 + hw])
                        xa_sb.append(t_)
                for tl in range(hw // P):
                    t = (h0 // P) + tl
                    ps = psproj.tile([P, F], F32, tag="projpsv", name="projpsv")
                    for e in range(EC):
                        for ns, nw in _cs(F, 512):
                            nc.tensor.matmul(
                                ps[:, ns:ns + nw],
                                xa_sb[e][:, tl * P:(tl + 1) * P],
                                wv_sb[e][:, ns:ns + nw],
                                start=(e == 0), stop=(e == EC - 1),
                            )
                    nc.vector.tensor_copy(v_sb[t][:], ps[:, 0:F])

        # ============ phase K: K-proj + RoPE =============================
        ktp = es_kqv.enter_context(tc.tile_pool(name="kt", bufs=1, side="right"))
        kt_sb = [ktp.tile([P, cT], BF, tag=f"kt{m}", name=f"kt{m}")
                 for m in range(HL)]
        # 448-wide segments divide the trimmed key length evenly (a 256-wide
        # tail segment would expose LDWEIGHTS behind short streams)
        SEGK = 448 if cT % 448 == 0 else SEG
        with tc.tile_pool(name="rawk", bufs=1) as rawkp, \
                tc.tile_pool(name="tmpk", bufs=2) as tmpkp:
            first = True
            for h0, hw in _cs(cT, SEGK):
                xa_sb = []
                for e in range(EC):
                    t_ = xakp.tile([P, SEG], BF, tag=f"xak{e}", name=f"xak{e}")
                    nc.sync.dma_start(
                        t_[:, 0:hw], xat_d[e * P:(e + 1) * P, h0:h0 + hw])
                    xa_sb.append(t_)
                if first:
                    # prefetch Q weights+tables behind seg-0 loads
                    first = False
                    cosq_sb = tabq.tile([P, cTQ], BF, tag="cosq", name="cosq")
                    sinq_sb = tabq.tile([P, cTQ], BF, tag="sinq", name="sinq")
                    nc.sync.dma_start(cosq_sb[:], cosq_d[:])
                    nc.sync.dma_start(sinq_sb[:], sinq_d[:])
                    wq_sb = []
                    for e in range(EC):
                        t_ = wqp.tile([P, F], BF, tag=f"wq{e}", name=f"wq{e}")
                        nc.sync.dma_start(t_[:], wq_d[e * P:(e + 1) * P, :])
                        wq_sb.append(t_)
                for m in range(HL):
                    raw = rawkp.tile([P, hw], BF, tag="rawk", name="rawk")
                    swp = rawkp.tile([P, hw], BF, tag="swpk", name="swpk")
                    ps = psproj.tile([P, SEG], F32, tag="projps", name="projps")
                    for e in range(EC):
                        nc.tensor.matmul(
                            ps[:, 0:hw],
                            wk_sb[e][:, m * P:(m + 1) * P],
                            xa_sb[e][:, 0:hw],
                            start=(e == 0), stop=(e == EC - 1),
                        )
                    nc.scalar.copy(raw[:], ps[:, 0:hw])
                    half = P // 2
                    nc.sync.dma_start(swp[0:half, :], raw[half:P, :])
                    nc.sync.dma_start(swp[half:P, :], raw[0:half, :])
                    t1 = tmpkp.tile([P, hw], BF, tag="rope_t1", name="rope_t1")
                    t2 = tmpkp.tile([P, hw], BF, tag="rope_t2", name="rope_t2")
                    nc.vector.tensor_mul(t1[:], raw[:],
                                         cosk_sb[:, h0:h0 + hw])
                    nc.vector.tensor_mul(t2[:], swp[:],
                                         sink_sb[:, h0:h0 + hw])
                    nc.vector.tensor_add(kt_sb[m][:, h0:h0 + hw], t1[:], t2[:])
            # prefetch the first x^T segment for the fused Q/attention phase
            xt0_sb = []
            for e in range(EC):
                t_ = xtp.tile([P, NQ], BF, tag=f"xt{e}", name=f"xt{e}")
                nc.sync.dma_start(t_[:], xt_d[e * P:(e + 1) * P, 0:NQ])
                xt0_sb.append(t_)
        es_xak.close()
        es_tabk.close()
        es_proj.close()

        # ====== phase C: fused Q-projection + attention, q-block sweeps ===
        es_wo = ExitStack()     # out-proj weights, loaded during attention
        wop = es_wo.enter_context(tc.tile_pool(name="wo", bufs=1))
        es_att = ExitStack()
        qtqp = es_att.enter_context(tc.tile_pool(name="qtq", bufs=2))
        rawqp = es_att.enter_context(tc.tile_pool(name="rawq", bufs=1))
        tmpqp = es_att.enter_context(tc.tile_pool(name="tmpq", bufs=2))
        ptp = es_att.enter_context(tc.tile_pool(name="pt", bufs=3))
        pt2p = es_att.enter_context(tc.tile_pool(name="pt2", bufs=4))
        accp = es_att.enter_context(tc.tile_pool(name="acc", bufs=2))
        ytqp = es_att.enter_context(tc.tile_pool(name="ytq", bufs=4))
        dstp = es_att.enter_context(tc.tile_pool(name="dst", bufs=3))
        pss = es_att.enter_context(tc.tile_pool(name="pss", bufs=2, space="PSUM"))
        psy = es_att.enter_context(tc.tile_pool(name="psy", bufs=2, space="PSUM"))
        psb = es_att.enter_context(tc.tile_pool(name="psb", bufs=2, space="PSUM"))

        wo_sb = [None] * (2 * F // P)
        GW = 2                    # score chunks per exp group (PSUM-limited)
        GR = [(i, min(i + GW, TC)) for i in range(0, TC, GW)]

        def stage_a(p):
            # denominator ones-matmul + psum evacuations + reciprocal
            dps = psb.tile([P, NQ], F32, tag="dpsbc", name="dps")
            nc.tensor.matmul(
                dps[0:1, :], ones_bf[:, 0:1], p["acc"][:],
                start=True, stop=True,
            )
            ytq = ytqp.tile([P, NQ], BF, tag="ytq", name="ytq")
            nc.vector.tensor_copy(ytq[:], p["yps"][:])
            dst = dstp.tile([1, NQ], F32, tag="dst", name="dst")
            nc.vector.tensor_copy(dst[0:1, :], dps[0:1, :])
            nc.vector.reciprocal(dst[0:1, :], dst[0:1, :])
            # DMA hop: the fp32r broadcast matmul requires a producer the
            # verifier accepts as rounded; DMA output qualifies
            dst2 = dstp.tile([1, NQ], F32, tag="dst2", name="dst2")
            nc.sync.dma_start(dst2[0:1, :], dst[0:1, :])
            p["ytq"], p["dst"] = ytq, dst2

        def stage_b(p):
            # broadcast 1/den across partitions, normalize, ship to pair
            dbc = psb.tile([P, NQ], F32, tag="dpsbc", name="dbc")
            nc.tensor.matmul(
                dbc[:, :],
                ones_fr[0:1, :].bitcast(FR),
                p["dst"][0:1, :].bitcast(FR),
                start=True, stop=True,
            )
            nc.vector.tensor_mul(p["ytq"][:], p["ytq"][:], dbc[:, :])
            m, qs = p["m"], p["qs"]
            blk, ml = divmod(m, 2)
            h, co = divmod(qs, HQ)
            j = 2 * blk + h
            nc.sync.dma_start(agin[j][ml * P:(ml + 1) * P, co:co + NQ],
                              p["ytq"][:])
            if ml == 1 and co + NQ == HQ:
                nc.gpsimd.collective_compute(
                    "AllGather", mybir.AluOpType.bypass,
                    replica_groups=groups,
                    ins=[agin[j][:]], outs=[agout[j][:]],
                )

        def finish_group(g, st, m):
            # exp + mask + P@V + denominator adds for score group g
            c0, c1 = GR[g]
            w = (c1 - c0) * NQ
            sps = st["sps"][g]
            pt = ptp.tile([P, GW * NQ], BF, tag="pt", name="pt")
            nc.scalar.activation(
                pt[:, 0:w], sps[:, 0:w],
                mybir.ActivationFunctionType.Exp, scale=SCALE,
            )
            for kc in range(max(c0, TC - NMASK), c1):
                j = kc - c0
                nc.vector.tensor_scalar_mul(
                    pt[:, j * NQ:(j + 1) * NQ],
                    pt[:, j * NQ:(j + 1) * NQ],
                    mb_sb[:, kc:kc + 1],
                )
            for kc in range(c0, c1):
                j = kc - c0
                nc.tensor.matmul(
                    st["yps"][:, :],
                    v_sb[kc][:, m * P:(m + 1) * P],
                    pt[:, j * NQ:(j + 1) * NQ],
                    start=(kc == 0), stop=(kc == TC - 1),
                )
            for j0 in range(0, c1 - c0, 2):
                pt2 = pt2p.tile([P, NQ], BF, tag="pt2", name="pt2")
                nc.vector.tensor_add(pt2[:], pt[:, j0 * NQ:(j0 + 1) * NQ],
                                     pt[:, (j0 + 1) * NQ:(j0 + 2) * NQ])
                if st["acc"] is None:
                    st["acc"] = pt2
                else:
                    nacc = accp.tile([P, NQ], BF, tag="acc", name="acc")
                    nc.vector.tensor_add(nacc[:], st["acc"][:], pt2[:])
                    st["acc"] = nacc

        pend = []                 # blocks awaiting stage A (last) / B (first)
        qsl = _cs(cTQ, NQ)
        for qsi, (qs, qw) in enumerate(qsl):
            assert qw == NQ
            # ---- Q-projection + RoPE for this q-block, all heads ----
            xt_sb = xt0_sb if qsi == 0 else nxt_sb
            cur_qt = []
            for m in range(HL):
                qtq = qtqp.tile([P, NQ], BF, tag=f"qtq{m}", name=f"qtq{m}")
                cur_qt.append(qtq)
                ps = pss.tile([P, GW * NQ], F32, tag="sps", name="qps")
                for e in range(EC):
                    nc.tensor.matmul(
                        ps[:, 0:NQ],
                        wq_sb[e][:, m * P:(m + 1) * P],
                        xt_sb[e][:, 0:NQ],
                        start=(e == 0), stop=(e == EC - 1),
                    )
                raw = rawqp.tile([P, NQ], BF, tag="rawq", name="rawq")
                swp = rawqp.tile([P, NQ], BF, tag="swpq", name="swpq")
                nc.vector.tensor_copy(raw[:], ps[:, 0:NQ])
                half = P // 2
                nc.sync.dma_start(swp[0:half, :], raw[half:P, :])
                nc.sync.dma_start(swp[half:P, :], raw[0:half, :])
                t1 = tmpqp.tile([P, NQ], BF, tag="rope_t1", name="rope_t1")
                t2 = tmpqp.tile([P, NQ], BF, tag="rope_t2", name="rope_t2")
                nc.vector.tensor_mul(t1[:], raw[:], cosq_sb[:, qs:qs + NQ])
                nc.vector.tensor_mul(t2[:], swp[:], sinq_sb[:, qs:qs + NQ])
                nc.vector.tensor_add(qtq[:], t1[:], t2[:])
            # prefetch next sweep's x^T segment
            if qsi + 1 < len(qsl):
                ns_, _ = qsl[qsi + 1]
                nxt_sb = []
                for e in range(EC):
                    t_ = xtp.tile([P, NQ], BF, tag=f"xt{e}", name=f"xt{e}")
                    nc.sync.dma_start(
                        t_[:], xt_d[e * P:(e + 1) * P, ns_:ns_ + NQ])
                    nxt_sb.append(t_)
            # ---- attention blocks for this q-block ----
            for m in range(HL):
                if qsi == 0 and 1 <= m <= 4:
                    # out-proj weights, 4 tiles per block: spread so these
                    # bulk DMAs never block the small denominator DMAs
                    for f in range(4 * (m - 1), 4 * m):
                        t_ = wop.tile([P, EH], BF, tag=f"wo{f}", name=f"wo{f}")
                        nc.sync.dma_start(t_[:], wo_d[f * P:(f + 1) * P, :])
                        wo_sb[f] = t_
                st = {
                    "yps": psy.tile([P, NQ], F32, tag="yps", name="yps"),
                    "sps": [],
                    "acc": None,
                    "m": m,
                    "qs": qs,
                }
                for g, (c0, c1) in enumerate(GR):
                    sps = pss.tile([P, GW * NQ], F32, tag="sps", name="sps")
                    st["sps"].append(sps)
                    for kc in range(c0, c1):
                        j = kc - c0
                        nc.tensor.matmul(
                            sps[:, j * NQ:(j + 1) * NQ],
                            kt_sb[m][:, kc * P:(kc + 1) * P],
                            cur_qt[m][:],
                            start=True, stop=True,
                        )
                    if g >= 1:
                        finish_group(g - 1, st, m)
                    if g == 1:
                        # older blocks' denominator work goes on the queues
                        # here, behind this block's first matmuls
                        if len(pend) == 2:
                            stage_b(pend.pop(0))
                        if pend:
                            stage_a(pend[0])
                finish_group(len(GR) - 1, st, m)
                pend.append(st)
        stage_a(pend[1])
        stage_b(pend[0])
        stage_b(pend[1])
        es_att.close()
        es_kqv.close()

        # ================= phase D: out-projection =======================
        # out^T[EH, q] = Wo'^T @ ya. For the second q-half the f contraction
        # is emitted in two sweeps over all 8 PSUM banks — f 0..11 first —
        # so ~20us of matmuls hide the final AllGathers and their loads.
        NT = EH // P
        NF = 2 * HL
        with tc.tile_pool(name="ya", bufs=1) as yap, \
                tc.tile_pool(name="oev", bufs=4) as oevp, \
                tc.tile_pool(name="pso", bufs=8, space="PSUM") as pso:
            ya_sb = []
            for f in range(NF):
                t_ = yap.tile([P, cTQ], BF, tag=f"ya{f}", name=f"ya{f}")
                ya_sb.append(t_)
            for h in range(2):
                for f in range(NF):
                    blk, r = divmod(f, 4)
                    nc.sync.dma_start(
                        ya_sb[f][:, h * HQ:(h + 1) * HQ],
                        agout[2 * blk + h][r * P:(r + 1) * P, :])
            for ms, mw in _cs(cTQ, 512):
                sweeps = [(0, NF)] if ms < HQ else [(0, 12), (12, NF)]
                opss = []
                for si, (f0, f1) in enumerate(sweeps):
                    for n in range(NT):
                        if si == 0:
                            ops = pso.tile([P, 512], F32, tag="ops", name="ops")
                            opss.append(ops)
                        else:
                            ops = opss[n]
                        for f in range(f0, f1):
                            nc.tensor.matmul(
                                ops[:, 0:mw],
                                wo_sb[f][:, n * P:(n + 1) * P],
                                ya_sb[f][:, ms:ms + mw],
                                start=(f == 0), stop=(f == NF - 1),
                            )
                        if si == len(sweeps) - 1:
                            oev = oevp.tile([P, 512], BF, tag="oev", name="oev")
                            if n % 2 == 0:
                                nc.scalar.copy(oev[:, 0:mw], ops[:, 0:mw])
                            else:
                                nc.vector.tensor_copy(oev[:, 0:mw],
                                                      ops[:, 0:mw])
                            nc.sync.dma_start(
                                out_d[n * P:(n + 1) * P, ms:ms + mw],
                                oev[:, 0:mw])
        es_wo.close()
        es_xt.close()
        es_tabq.close()

    return nc


# ---------------------------------------------------------------------------
# host side
# ---------------------------------------------------------------------------

def _rope_tables():
    inv_freq = 1.0 / (THETA ** (np.arange(0, D, 2, dtype=np.float32) / D))
    t = np.arange(BLOCK, dtype=np.float32)
    freqs = np.einsum("i,j->ij", t, inv_freq).astype(np.float32)
    emb = np.concatenate([freqs, freqs], axis=-1)
    return np.cos(emb).astype(np.float32), np.sin(emb).astype(np.float32)


_NC_CACHE = {}


def _get_compiled():
    if "nc" not in _NC_CACHE:
        nc = build_nc()
        nc.compile()
        _NC_CACHE["nc"] = nc
    return _NC_CACHE["nc"]


def _bf(a):
    return np.ascontiguousarray(a).astype(BF16NP)


def prepare_in_maps(x, xall, posx, posxall, mask, Wq, Wk, Wv, Wo):
    x = np.asarray(x, dtype=np.float32)
    xall = np.asarray(xall, dtype=np.float32)
    posx = np.asarray(posx)
    posxall = np.asarray(posxall)
    mask = np.asarray(mask)
    Wq = np.asarray(Wq, dtype=np.float32)
    Wk = np.asarray(Wk, dtype=np.float32)
    Wv = np.asarray(Wv, dtype=np.float32)
    Wo = np.asarray(Wo, dtype=np.float32)

    cos_t, sin_t = _rope_tables()
    sign = np.ones((1, D), np.float32)
    sign[0, : D // 2] = -1.0

    F = (H * D) // 2  # 1024: per-core head-shard width
    FB = 2 * D        # 256: AllGather block (2 heads)
    # AllGather block order: [A blk_i, B blk_i] for i in 0..3, where A/B are
    # the pair's rank-0/rank-1 feature halves of Wo's rows
    Wo_perm = np.concatenate(
        [w for i in range(4)
         for w in (Wo[i * FB:(i + 1) * FB], Wo[F + i * FB:F + (i + 1) * FB])],
        axis=0)

    in_maps = []
    for c in range(N_CORES):
        b, hg = c // 2, c % 2
        sl = slice(hg * F, (hg + 1) * F)
        # sort keys unmasked-first (stable) and keep the first KA: every
        # dropped key is masked (zero attention weight) so this is exact;
        # remaining masked keys land in the last NMASK chunks
        order = np.argsort(mask[b], kind="stable")[:KA]
        act = int((~mask[b]).sum())
        assert act <= KA, f"batch {b}: {act} active keys > {KA}"
        assert KA - act <= NMASK * P, \
            f"batch {b}: masked keys spill out of the last {NMASK} chunks"
        xall_p = xall[b][order]
        posk_p = posxall[b][order]
        mask_p = mask[b][order]
        cosq = _bf(cos_t[posx[b]].T)                    # [128, TQ]
        sinq = _bf((sin_t[posx[b]] * sign).T)
        cosk = _bf(cos_t[posk_p].T)                     # [128, KA]
        sink = _bf((sin_t[posk_p] * sign).T)
        m01 = np.where(mask_p, np.float32(0.0), np.float32(1.0))
        m01 = np.ascontiguousarray(m01.reshape(KA // P, P).T)  # [128, TCA]
        in_maps.append({
            "xt": _bf(x[b].T),
            "xat": _bf(xall_p.T),
            "wq": _bf(Wq[:, sl]),
            "wk": _bf(Wk[:, sl]),
            "wv": _bf(Wv[:, sl]),
            "wo": _bf(Wo_perm[:, hg * (E // 2):(hg + 1) * (E // 2)]),
            "cosq": cosq, "sinq": sinq, "cosk": cosk, "sink": sink,
            "mask01": m01.astype(np.float32),
        })
    return in_maps


def assemble_out(results):
    # core (b, hg) computed out^T for E columns [hg*E/2, (hg+1)*E/2)
    EH = E // 2
    out = np.empty((B, TQ, E), np.float32)
    for b in range(B):
        for hg in range(2):
            half = results[2 * b + hg]["out"].astype(np.float32)
            out[b][:, hg * EH:(hg + 1) * EH] = half.T
    return out


def kernel(x, xall, posx, posxall, mask, Wq, Wk, Wv, Wo):
    from concourse.bass_utils import run_bass_kernel_spmd

    in_maps = prepare_in_maps(x, xall, posx, posxall, mask, Wq, Wk, Wv, Wo)
    nc = _get_compiled()
    res = run_bass_kernel_spmd(nc, in_maps, list(range(N_CORES)), trace=False)
    return assemble_out(res.results)


# revision 34
# speedup vs baseline: 1.1166x; 1.0139x over previous
"""Distributed Trainium2 Bass kernel for nn_Attention_25460566131147.

Multi-head attention (B=4, TQ=T=2048, E=2048, H=16, D=128) with gather-based
RoPE and key masking, sharded over 8 NeuronCores: data-parallel over batch
(4 groups) x tensor-parallel over heads (2-way: Wq/Wk/Wv column shards).

Structure (v4):
  - keys are sorted unmasked-first on the host (softmax is permutation-
    invariant over keys) and truncated to KA=1792: dropped keys are all
    masked (zero attention weight) so the result is exact. Remaining masked
    keys land in the last NMASK chunks and are zeroed after exp via a
    per-partition mask multiply; every other chunk needs no mask, letting
    exp run as wide bias-free activations (ACT costs (N+352)/1.2 ns).
  - the Q projection is FUSED into the attention phase: attention runs
    q-block-outer / head-inner, and each sweep first projects+RoPEs its own
    512-wide q-block for all heads. Attention is ScalarE(exp)-bound while
    projections are TensorE-bound, so fusing fills each engine's bubbles.
  - softmax normalization is per (head, q-block) via a two-stage pipelined
    flush (denominator ones-matmul + reciprocal one block later, broadcast
    multiply + ship one block after that) so the in-order TensorE queue
    never waits on the VectorE chain.
  - normalized yt q-slices are AllGathered within the pair in 8 pieces
    (head-pair x q-half) as they complete; the out-projection contracts the
    gathered blocks with a host-permuted Wo (rank-independent) computing
    this core's E-half, ordered so the final AllGathers are hidden behind
    matmuls on already-arrived data.
  - scores are computed transposed (S^T[k,q]) so the exp'd tile feeds P@V
    directly; the softmax denominator comes from a pairwise add + running
    accumulator on VectorE plus a single ones-column matmul (GpSimd is
    avoided: each collective trigger blocks its queue ~20us).
  - a PE warm-up matmul chain covers the initial DMA wait so the HAM clock
    gate reaches 2.4GHz before real work; phase weights/tables are
    prefetched a phase ahead (SBUF pools are strict LIFO per side).
"""

import os
import sys

if "JAX_PLATFORMS" in os.environ and os.environ["JAX_PLATFORMS"] == "axon":
    os.environ["JAX_PLATFORMS"] = "axon,cpu"
sys.path.insert(0, "/opt/trn_rl_repo")

import numpy as np
import ml_dtypes

BF16NP = ml_dtypes.bfloat16

B, TQ, T, E, H, D = 4, 2048, 2048, 2048, 16, 128
BLOCK, THETA = 4096, 10000.0
N_CORES = 8
P = 128

KA = 14 * P               # 1792 active keys kept per batch
NMASK = 2                 # trailing chunks that receive the mask multiply

FULL_CFG = dict(TQ=TQ, T=KA, E=E, HL=8, D=D, NCORES=N_CORES)


def _cs(total, w):
    """Column splits: list of (start, width)."""
    return [(i, min(w, total - i)) for i in range(0, total, w)]


def build_nc(cfg=None):
    """Build and return the (uncompiled) Bacc graph for one SPMD core."""
    import concourse.mybir as mybir
    import concourse.tile as tile
    from concourse import bacc
    from contextlib import ExitStack

    c = dict(FULL_CFG)
    if cfg:
        c.update(cfg)
    cTQ, cT, cE, HL, cD, NCORES = (
        c["TQ"], c["T"], c["E"], c["HL"], c["D"], c["NCORES"],
    )
    assert cD == P
    F = HL * cD              # local feature width (heads shard)
    EC = cE // P             # contraction chunks for projections
    TC = cT // P             # active key chunks (14)
    NQ = 512                 # q-block width
    HQ = cTQ // 2            # q-half width (AllGather granule)
    EH = cE // 2             # out-feature half owned by this core
    BF = mybir.dt.bfloat16
    F32 = mybir.dt.float32
    FR = mybir.dt.float32r
    SCALE = 1.0 / float(np.sqrt(cD))
    groups = [[2 * i, 2 * i + 1] for i in range(NCORES // 2)]

    nc = bacc.Bacc("TRN2", target_bir_lowering=False, debug=False,
                   num_devices=NCORES)

    xt_d = nc.declare_dram_parameter("xt", [cE, cTQ], BF, isOutput=False)
    xat_d = nc.declare_dram_parameter("xat", [cE, cT], BF, isOutput=False)
    wq_d = nc.declare_dram_parameter("wq", [cE, F], BF, isOutput=False)
    wk_d = nc.declare_dram_parameter("wk", [cE, F], BF, isOutput=False)
    wv_d = nc.declare_dram_parameter("wv", [cE, F], BF, isOutput=False)
    # host-permuted Wo rows (AllGather block order), this core's E-col half
    wo_d = nc.declare_dram_parameter("wo", [2 * F, EH], BF, isOutput=False)
    cosq_d = nc.declare_dram_parameter("cosq", [P, cTQ], BF, isOutput=False)
    sinq_d = nc.declare_dram_parameter("sinq", [P, cTQ], BF, isOutput=False)
    cosk_d = nc.declare_dram_parameter("cosk", [P, cT], BF, isOutput=False)
    sink_d = nc.declare_dram_parameter("sink", [P, cT], BF, isOutput=False)
    mb_d = nc.declare_dram_parameter("mask01", [P, TC], F32, isOutput=False)
    out_d = nc.declare_dram_parameter("out", [EH, cTQ], BF, isOutput=True)

    # yt exchange: 8 buffers = (head-pair block) x (q half)
    NBLK = HL // 2
    agin = [nc.dram_tensor(f"agin{j}", [2 * P, HQ], BF) for j in range(8)]
    agout = [nc.dram_tensor(f"agout{j}", [4 * P, HQ], BF) for j in range(8)]

    with tile.TileContext(nc) as tc, ExitStack() as ex:
        consts = ex.enter_context(tc.tile_pool(name="consts", bufs=1, side="right"))
        ones_bf = consts.tile([P, 1], BF, tag="ones_bf", name="ones_bf")
        nc.vector.memset(ones_bf[:], 1.0)
        mb_sb = consts.tile([P, TC], F32, tag="mask01", name="mask01")
        nc.sync.dma_start(mb_sb[:], mb_d[:])
        ones_fr = consts.tile([1, P], F32, tag="ones_fr", name="ones_fr")
        nc.vector.memset(ones_fr[:], 1.0)

        vp = ex.enter_context(tc.tile_pool(name="v", bufs=1, side="right"))
        es_kqv = ExitStack()  # kt pool: closed before phase D (SBUF reuse)

        # left-side pool stack, opened in reverse close order (LIFO):
        es_proj = ExitStack()   # V/K projection psum: [V .. K]
        warmp = es_proj.enter_context(
            tc.tile_pool(name="warm", bufs=1, space="PSUM"))
        psproj = es_proj.enter_context(
            tc.tile_pool(name="psproj", bufs=2, space="PSUM"))
        es_tabq = ExitStack()   # Q weights+tables: [pre-V .. end]
        wqp = es_tabq.enter_context(tc.tile_pool(name="wq", bufs=1))
        tabq = es_tabq.enter_context(tc.tile_pool(name="tabq", bufs=1))
        es_xt = ExitStack()     # x^T quarters, double-buffered: [pre-V .. end]
        xtp = es_xt.enter_context(tc.tile_pool(name="xt", bufs=2))
        es_tabk = ExitStack()   # K weights+tables: [pre-V .. K]
        wkp = es_tabk.enter_context(tc.tile_pool(name="wk", bufs=1))
        tabk = es_tabk.enter_context(tc.tile_pool(name="tabk", bufs=1))
        es_xak = ExitStack()    # xall^T quarters for K: [pre-V .. K]
        xakp = es_xak.enter_context(tc.tile_pool(name="xak", bufs=1))

        SEG = min(512, cT)   # projection column-segment width

        # ============ phase V: V = xall @ Wv, [t-part, n-free] ===========
        assert F <= 1024
        v_sb = [vp.tile([P, F], BF, tag=f"v{t}", name=f"v{t}")
                for t in range(TC)]
        with tc.tile_pool(name="wv", bufs=1) as wvp, \
                tc.tile_pool(name="xav", bufs=1) as xavp:
            wv_sb = []
            for e in range(EC):
                t_ = wvp.tile([P, F], BF, tag=f"wv{e}", name=f"wv{e}")
                wv_sb.append(t_)
            # critical first tiles first, then the rest
            seg0_xa = []
            for e in range(EC):
                t_ = xavp.tile([P, SEG], BF, tag=f"xav{e}", name=f"xav{e}")
                nc.sync.dma_start(t_[:], xat_d[e * P:(e + 1) * P, 0:SEG])
                seg0_xa.append(t_)
            nc.sync.dma_start(wv_sb[0][:], wv_d[0:P, :])
            for e in range(1, EC):
                nc.sync.dma_start(wv_sb[e][:], wv_d[e * P:(e + 1) * P, :])
            # PE warm-up chain: matmuls with no input deps keep the HAM
            # activity monitor busy during the initial DMA wait so the
            # first real matmuls run at 2.4GHz instead of 1.2GHz
            dumw = wvp.tile([P, 512], BF, tag="dumw", name="dumw")
            nc.vector.memset(dumw[:], 0.0)
            wps = warmp.tile([1, 512], F32, tag="wps", name="wps")
            for _ in range(48):
                nc.tensor.matmul(wps[0:1, :], ones_bf[:, 0:1], dumw[:],
                                 start=True, stop=True)
            # prefetch K-phase tables+weights (used next phase)
            cosk_sb = tabk.tile([P, cT], BF, tag="cosk", name="cosk")
            sink_sb = tabk.tile([P, cT], BF, tag="sink", name="sink")
            nc.sync.dma_start(cosk_sb[:], cosk_d[:])
            nc.sync.dma_start(sink_sb[:], sink_d[:])
            wk_sb = []
            for e in range(EC):
                t_ = wkp.tile([P, F], BF, tag=f"wk{e}", name=f"wk{e}")
                nc.sync.dma_start(t_[:], wk_d[e * P:(e + 1) * P, :])
                wk_sb.append(t_)
            for h0, hw in _cs(cT, SEG):
                if h0 == 0:
                    xa_sb = seg0_xa
                else:
                    xa_sb = []
                    for e in range(EC):
                        t_ = xavp.tile([P, SEG], BF, tag=f"xav{e}", name=f"xav{e}")
                        nc.sync.dma_start(
                            t_[:, 0:hw], xat_d[e * P:(e + 1) * P, h0:h0 + hw])
                        xa_sb.append(t_)
                for tl in range(hw // P):
                    t = (h0 // P) + tl
                    ps = psproj.tile([P, F], F32, tag="projpsv", name="projpsv")
                    for e in range(EC):
                        for ns, nw in _cs(F, 512):
                            nc.tensor.matmul(
                                ps[:, ns:ns + nw],
                                xa_sb[e][:, tl * P:(tl + 1) * P],
                                wv_sb[e][:, ns:ns + nw],
                                start=(e == 0), stop=(e == EC - 1),
                            )
                    nc.vector.tensor_copy(v_sb[t][:], ps[:, 0:F])

        # ============ phase K: K-proj + RoPE =============================
        ktp = es_kqv.enter_context(tc.tile_pool(name="kt", bufs=1, side="right"))
        kt_sb = [ktp.tile([P, cT], BF, tag=f"kt{m}", name=f"kt{m}")
                 for m in range(HL)]
        # 448-wide segments divide the trimmed key length evenly (a 256-wide
        # tail segment would expose LDWEIGHTS behind short streams)
        SEGK = 448 if cT % 448 == 0 else SEG
        with tc.tile_pool(name="rawk", bufs=1) as rawkp, \
                tc.tile_pool(name="tmpk", bufs=2) as tmpkp:
            first = True
            for h0, hw in _cs(cT, SEGK):
                xa_sb = []
                for e in range(EC):
                    t_ = xakp.tile([P, SEG], BF, tag=f"xak{e}", name=f"xak{e}")
                    nc.sync.dma_start(
                        t_[:, 0:hw], xat_d[e * P:(e + 1) * P, h0:h0 + hw])
                    xa_sb.append(t_)
                if first:
                    # prefetch Q weights+tables behind seg-0 loads
                    first = False
                    cosq_sb = tabq.tile([P, cTQ], BF, tag="cosq", name="cosq")
                    sinq_sb = tabq.tile([P, cTQ], BF, tag="sinq", name="sinq")
                    nc.sync.dma_start(cosq_sb[:], cosq_d[:])
                    nc.sync.dma_start(sinq_sb[:], sinq_d[:])
                    wq_sb = []
                    for e in range(EC):
                        t_ = wqp.tile([P, F], BF, tag=f"wq{e}", name=f"wq{e}")
                        nc.sync.dma_start(t_[:], wq_d[e * P:(e + 1) * P, :])
                        wq_sb.append(t_)
                for m in range(HL):
                    raw = rawkp.tile([P, hw], BF, tag="rawk", name="rawk")
                    swp = rawkp.tile([P, hw], BF, tag="swpk", name="swpk")
                    ps = psproj.tile([P, SEG], F32, tag="projps", name="projps")
                    for e in range(EC):
                        nc.tensor.matmul(
                            ps[:, 0:hw],
                            wk_sb[e][:, m * P:(m + 1) * P],
                            xa_sb[e][:, 0:hw],
                            start=(e == 0), stop=(e == EC - 1),
                        )
                    nc.scalar.copy(raw[:], ps[:, 0:hw])
                    half = P // 2
                    nc.sync.dma_start(swp[0:half, :], raw[half:P, :])
                    nc.sync.dma_start(swp[half:P, :], raw[0:half, :])
                    t1 = tmpkp.tile([P, hw], BF, tag="rope_t1", name="rope_t1")
                    t2 = tmpkp.tile([P, hw], BF, tag="rope_t2", name="rope_t2")
                    nc.vector.tensor_mul(t1[:], raw[:],
                                         cosk_sb[:, h0:h0 + hw])
                    nc.vector.tensor_mul(t2[:], swp[:],
                                         sink_sb[:, h0:h0 + hw])
                    nc.vector.tensor_add(kt_sb[m][:, h0:h0 + hw], t1[:], t2[:])
            # prefetch the first x^T segment for the fused Q/attention phase
            xt0_sb = []
            for e in range(EC):
                t_ = xtp.tile([P, NQ], BF, tag=f"xt{e}", name=f"xt{e}")
                nc.sync.dma_start(t_[:], xt_d[e * P:(e + 1) * P, 0:NQ])
                xt0_sb.append(t_)
        es_xak.close()
        es_tabk.close()
        es_proj.close()

        # ====== phase C: fused Q-projection + attention, q-block sweeps ===
        es_wo = ExitStack()     # out-proj weights, loaded during attention
        wop = es_wo.enter_context(tc.tile_pool(name="wo", bufs=1))
        es_att = ExitStack()
        qtqp = es_att.enter_context(tc.tile_pool(name="qtq", bufs=2))
        rawqp = es_att.enter_context(tc.tile_pool(name="rawq", bufs=1))
        tmpqp = es_att.enter_context(tc.tile_pool(name="tmpq", bufs=1))
        ptp = es_att.enter_context(tc.tile_pool(name="pt", bufs=3))
        pt2p = es_att.enter_context(tc.tile_pool(name="pt2", bufs=4))
        accp = es_att.enter_context(tc.tile_pool(name="acc", bufs=2))
        ytqp = es_att.enter_context(tc.tile_pool(name="ytq", bufs=4))
        dstp = es_att.enter_context(tc.tile_pool(name="dst", bufs=2))
        pss = es_att.enter_context(tc.tile_pool(name="pss", bufs=2, space="PSUM"))
        psy = es_att.enter_context(tc.tile_pool(name="psy", bufs=2, space="PSUM"))
        psb = es_att.enter_context(tc.tile_pool(name="psb", bufs=2, space="PSUM"))

        wo_sb = [None] * (2 * F // P)
        GW = 2                    # score chunks per exp group (PSUM-limited)
        GR = [(i, min(i + GW, TC)) for i in range(0, TC, GW)]

        def stage_a(p):
            # denominator ones-matmul + psum evacuations + reciprocal
            dps = psb.tile([P, NQ], F32, tag="dpsbc", name="dps")
            nc.tensor.matmul(
                dps[0:1, :], ones_bf[:, 0:1], p["acc"][:],
                start=True, stop=True,
            )
            ytq = ytqp.tile([P, NQ], BF, tag="ytq", name="ytq")
            nc.vector.tensor_copy(ytq[:], p["yps"][:])
            dst = dstp.tile([1, NQ], F32, tag="dst", name="dst")
            nc.vector.tensor_copy(dst[0:1, :], dps[0:1, :])
            nc.vector.reciprocal(dst[0:1, :], dst[0:1, :])
            # DMA hop: the fp32r broadcast matmul requires a producer the
            # verifier accepts as rounded; DMA output qualifies
            dst2 = dstp.tile([1, NQ], F32, tag="dst2", name="dst2")
            nc.sync.dma_start(dst2[0:1, :], dst[0:1, :])
            p["ytq"], p["dst"] = ytq, dst2

        def stage_b(p):
            # broadcast 1/den across partitions, normalize, ship to pair
            dbc = psb.tile([P, NQ], F32, tag="dpsbc", name="dbc")
            nc.tensor.matmul(
                dbc[:, :],
                ones_fr[0:1, :].bitcast(FR),
                p["dst"][0:1, :].bitcast(FR),
                start=True, stop=True,
            )
            nc.vector.tensor_mul(p["ytq"][:], p["ytq"][:], dbc[:, :])
            m, qs = p["m"], p["qs"]
            blk, ml = divmod(m, 2)
            h, co = divmod(qs, HQ)
            j = 2 * blk + h
            nc.sync.dma_start(agin[j][ml * P:(ml + 1) * P, co:co + NQ],
                              p["ytq"][:])
            if ml == 1 and co + NQ == HQ:
                nc.gpsimd.collective_compute(
                    "AllGather", mybir.AluOpType.bypass,
                    replica_groups=groups,
                    ins=[agin[j][:]], outs=[agout[j][:]],
                )

        def finish_group(g, st, m):
            # exp + mask + P@V + denominator adds for score group g
            c0, c1 = GR[g]
            w = (c1 - c0) * NQ
            sps = st["sps"][g]
            pt = ptp.tile([P, GW * NQ], BF, tag="pt", name="pt")
            nc.scalar.activation(
                pt[:, 0:w], sps[:, 0:w],
                mybir.ActivationFunctionType.Exp, scale=SCALE,
            )
            for kc in range(max(c0, TC - NMASK), c1):
                j = kc - c0
                nc.vector.tensor_scalar_mul(
                    pt[:, j * NQ:(j + 1) * NQ],
                    pt[:, j * NQ:(j + 1) * NQ],
                    mb_sb[:, kc:kc + 1],
                )
            for kc in range(c0, c1):
                j = kc - c0
                nc.tensor.matmul(
                    st["yps"][:, :],
                    v_sb[kc][:, m * P:(m + 1) * P],
                    pt[:, j * NQ:(j + 1) * NQ],
                    start=(kc == 0), stop=(kc == TC - 1),
                )
            for j0 in range(0, c1 - c0, 2):
                pt2 = pt2p.tile([P, NQ], BF, tag="pt2", name="pt2")
                nc.vector.tensor_add(pt2[:], pt[:, j0 * NQ:(j0 + 1) * NQ],
                                     pt[:, (j0 + 1) * NQ:(j0 + 2) * NQ])
                if st["acc"] is None:
                    st["acc"] = pt2
                else:
                    nacc = accp.tile([P, NQ], BF, tag="acc", name="acc")
                    nc.vector.tensor_add(nacc[:], st["acc"][:], pt2[:])
                    st["acc"] = nacc

        qsl = _cs(cTQ, NQ)

        def emit_qproj(m, qs0, xt_sb, dst_qt):
            # project+RoPE one head's q-block (raw copy on VectorE: the
            # ScalarE is the attention pacer)
            ps = pss.tile([P, GW * NQ], F32, tag="sps", name="qps")
            for e in range(EC):
                nc.tensor.matmul(
                    ps[:, 0:NQ],
                    wq_sb[e][:, m * P:(m + 1) * P],
                    xt_sb[e][:, 0:NQ],
                    start=(e == 0), stop=(e == EC - 1),
                )
            raw = rawqp.tile([P, NQ], BF, tag="rawq", name="rawq")
            swp = rawqp.tile([P, NQ], BF, tag="swpq", name="swpq")
            nc.vector.tensor_copy(raw[:], ps[:, 0:NQ])
            half = P // 2
            nc.sync.dma_start(swp[0:half, :], raw[half:P, :])
            nc.sync.dma_start(swp[half:P, :], raw[0:half, :])
            t1 = tmpqp.tile([P, NQ], BF, tag="rope_t1", name="rope_t1")
            t2 = tmpqp.tile([P, NQ], BF, tag="rope_t2", name="rope_t2")
            nc.vector.tensor_mul(t1[:], raw[:], cosq_sb[:, qs0:qs0 + NQ])
            nc.vector.tensor_mul(t2[:], swp[:], sinq_sb[:, qs0:qs0 + NQ])
            nc.vector.tensor_add(dst_qt[:], t1[:], t2[:])

        def load_xt(si):
            l = []
            for e in range(EC):
                t_ = xtp.tile([P, NQ], BF, tag=f"xt{e}", name=f"xt{e}")
                nc.sync.dma_start(
                    t_[:], xt_d[e * P:(e + 1) * P, qsl[si][0]:qsl[si][0] + NQ])
                l.append(t_)
            return l

        # prologue: project sweep 0 for all heads
        cur_qt = []
        for m in range(HL):
            qtq = qtqp.tile([P, NQ], BF, tag=f"qtq{m}", name=f"qtq{m}")
            emit_qproj(m, 0, xt0_sb, qtq)
            cur_qt.append(qtq)
        xt_next = load_xt(1)

        pend = []                 # blocks awaiting stage A (last) / B (first)
        prev_tail = None          # block whose last score group is unfinished
        for qsi, (qs, qw) in enumerate(qsl):
            assert qw == NQ
            xt_after = load_xt(qsi + 2) if qsi + 2 < len(qsl) else None
            nxt_qt = []
            for m in range(HL):
                if qsi + 1 < len(qsl):
                    # next sweep's Q-projection, interleaved per head: fills
                    # TensorE while ScalarE drains this block's exps
                    qtq = qtqp.tile([P, NQ], BF, tag=f"qtq{m}", name=f"qtq{m}")
                    emit_qproj(m, qsl[qsi + 1][0], xt_next, qtq)
                    nxt_qt.append(qtq)
                if qsi == 0 and 1 <= m <= 4:
                    # out-proj weights, 4 tiles per block: spread so these
                    # bulk DMAs never block the small denominator DMAs
                    for f in range(4 * (m - 1), 4 * m):
                        t_ = wop.tile([P, EH], BF, tag=f"wo{f}", name=f"wo{f}")
                        nc.sync.dma_start(t_[:], wo_d[f * P:(f + 1) * P, :])
                        wo_sb[f] = t_
                st = {
                    "yps": psy.tile([P, NQ], F32, tag="yps", name="yps"),
                    "sps": [],
                    "acc": None,
                    "m": m,
                    "qs": qs,
                }
                for g, (c0, c1) in enumerate(GR):
                    sps = pss.tile([P, GW * NQ], F32, tag="sps", name="sps")
                    st["sps"].append(sps)
                    for kc in range(c0, c1):
                        j = kc - c0
                        nc.tensor.matmul(
                            sps[:, j * NQ:(j + 1) * NQ],
                            kt_sb[m][:, kc * P:(kc + 1) * P],
                            cur_qt[m][:],
                            start=True, stop=True,
                        )
                    if g == 0 and prev_tail is not None:
                        # previous block's last group: deferred here so its
                        # exp-wait hides behind this block's matmuls instead
                        # of stalling the in-order TensorE queue
                        finish_group(len(GR) - 1, *prev_tail)
                        prev_tail = None
                    if g >= 1:
                        finish_group(g - 1, st, m)
                    if g == 1:
                        # older blocks' denominator work goes on the queues
                        # here, behind this block's first matmuls
                        if len(pend) == 2:
                            stage_b(pend.pop(0))
                        if pend:
                            stage_a(pend[0])
                prev_tail = (st, m)
                pend.append(st)
            cur_qt = nxt_qt if nxt_qt else cur_qt
            xt_next = xt_after
        finish_group(len(GR) - 1, *prev_tail)
        stage_b(pend.pop(0))
        stage_a(pend[0])
        stage_b(pend[0])
        es_att.close()
        es_kqv.close()

        # ================= phase D: out-projection =======================
        # out^T[EH, q] = Wo'^T @ ya. For the second q-half the f contraction
        # is emitted in two sweeps over all 8 PSUM banks — f 0..11 first —
        # so ~20us of matmuls hide the final AllGathers and their loads.
        NT = EH // P
        NF = 2 * HL
        with tc.tile_pool(name="ya", bufs=1) as yap, \
                tc.tile_pool(name="oev", bufs=4) as oevp, \
                tc.tile_pool(name="pso", bufs=8, space="PSUM") as pso:
            ya_sb = []
            for f in range(NF):
                t_ = yap.tile([P, cTQ], BF, tag=f"ya{f}", name=f"ya{f}")
                ya_sb.append(t_)
            for h in range(2):
                for f in range(NF):
                    blk, r = divmod(f, 4)
                    nc.sync.dma_start(
                        ya_sb[f][:, h * HQ:(h + 1) * HQ],
                        agout[2 * blk + h][r * P:(r + 1) * P, :])
            for ms, mw in _cs(cTQ, 512):
                sweeps = [(0, NF)] if ms < HQ else [(0, 12), (12, NF)]
                opss = []
                for si, (f0, f1) in enumerate(sweeps):
                    for n in range(NT):
                        if si == 0:
                            ops = pso.tile([P, 512], F32, tag="ops", name="ops")
                            opss.append(ops)
                        else:
                            ops = opss[n]
                        for f in range(f0, f1):
                            nc.tensor.matmul(
                                ops[:, 0:mw],
                                wo_sb[f][:, n * P:(n + 1) * P],
                                ya_sb[f][:, ms:ms + mw],
                                start=(f == 0), stop=(f == NF - 1),
                            )
                        if si == len(sweeps) - 1:
                            oev = oevp.tile([P, 512], BF, tag="oev", name="oev")
                            if n % 2 == 0:
                                nc.scalar.copy(oev[:, 0:mw], ops[:, 0:mw])
                            else:
                                nc.vector.tensor_copy(oev[:, 0:mw],
                                                      ops[:, 0:mw])
                            nc.sync.dma_start(
                                out_d[n * P:(n + 1) * P, ms:ms + mw],
                                oev[:, 0:mw])
        es_wo.close()
        es_xt.close()
        es_tabq.close()

    return nc


# ---------------------------------------------------------------------------
# host side
# ---------------------------------------------------------------------------

def _rope_tables():
    inv_freq = 1.0 / (THETA ** (np.arange(0, D, 2, dtype=np.float32) / D))
    t = np.arange(BLOCK, dtype=np.float32)
    freqs = np.einsum("i,j->ij", t, inv_freq).astype(np.float32)
    emb = np.concatenate([freqs, freqs], axis=-1)
    return np.cos(emb).astype(np.float32), np.sin(emb).astype(np.float32)


_NC_CACHE = {}


def _get_compiled():
    if "nc" not in _NC_CACHE:
        nc = build_nc()
        nc.compile()
        _NC_CACHE["nc"] = nc
    return _NC_CACHE["nc"]


def _bf(a):
    return np.ascontiguousarray(a).astype(BF16NP)


def prepare_in_maps(x, xall, posx, posxall, mask, Wq, Wk, Wv, Wo):
    x = np.asarray(x, dtype=np.float32)
    xall = np.asarray(xall, dtype=np.float32)
    posx = np.asarray(posx)
    posxall = np.asarray(posxall)
    mask = np.asarray(mask)
    Wq = np.asarray(Wq, dtype=np.float32)
    Wk = np.asarray(Wk, dtype=np.float32)
    Wv = np.asarray(Wv, dtype=np.float32)
    Wo = np.asarray(Wo, dtype=np.float32)

    cos_t, sin_t = _rope_tables()
    sign = np.ones((1, D), np.float32)
    sign[0, : D // 2] = -1.0

    F = (H * D) // 2  # 1024: per-core head-shard width
    FB = 2 * D        # 256: AllGather block (2 heads)
    # AllGather block order: [A blk_i, B blk_i] for i in 0..3, where A/B are
    # the pair's rank-0/rank-1 feature halves of Wo's rows
    Wo_perm = np.concatenate(
        [w for i in range(4)
         for w in (Wo[i * FB:(i + 1) * FB], Wo[F + i * FB:F + (i + 1) * FB])],
        axis=0)

    in_maps = []
    for c in range(N_CORES):
        b, hg = c // 2, c % 2
        sl = slice(hg * F, (hg + 1) * F)
        # sort keys unmasked-first (stable) and keep the first KA: every
        # dropped key is masked (zero attention weight) so this is exact;
        # remaining masked keys land in the last NMASK chunks
        order = np.argsort(mask[b], kind="stable")[:KA]
        act = int((~mask[b]).sum())
        assert act <= KA, f"batch {b}: {act} active keys > {KA}"
        assert KA - act <= NMASK * P, \
            f"batch {b}: masked keys spill out of the last {NMASK} chunks"
        xall_p = xall[b][order]
        posk_p = posxall[b][order]
        mask_p = mask[b][order]
        cosq = _bf(cos_t[posx[b]].T)                    # [128, TQ]
        sinq = _bf((sin_t[posx[b]] * sign).T)
        cosk = _bf(cos_t[posk_p].T)                     # [128, KA]
        sink = _bf((sin_t[posk_p] * sign).T)
        m01 = np.where(mask_p, np.float32(0.0), np.float32(1.0))
        m01 = np.ascontiguousarray(m01.reshape(KA // P, P).T)  # [128, TCA]
        in_maps.append({
            "xt": _bf(x[b].T),
            "xat": _bf(xall_p.T),
            "wq": _bf(Wq[:, sl]),
            "wk": _bf(Wk[:, sl]),
            "wv": _bf(Wv[:, sl]),
            "wo": _bf(Wo_perm[:, hg * (E // 2):(hg + 1) * (E // 2)]),
            "cosq": cosq, "sinq": sinq, "cosk": cosk, "sink": sink,
            "mask01": m01.astype(np.float32),
        })
    return in_maps


def assemble_out(results):
    # core (b, hg) computed out^T for E columns [hg*E/2, (hg+1)*E/2)
    EH = E // 2
    out = np.empty((B, TQ, E), np.float32)
    for b in range(B):
        for hg in range(2):
            half = results[2 * b + hg]["out"].astype(np.float32)
            out[b][:, hg * EH:(hg + 1) * EH] = half.T
    return out


def kernel(x, xall, posx, posxall, mask, Wq, Wk, Wv, Wo):
    from concourse.bass_utils import run_bass_kernel_spmd

    in_maps = prepare_in_maps(x, xall, posx, posxall, mask, Wq, Wk, Wv, Wo)
    nc = _get_compiled()
    res = run_bass_kernel_spmd(nc, in_maps, list(range(N_CORES)), trace=False)
    return assemble_out(res.results)


# revision 37
# speedup vs baseline: 1.1395x; 1.0205x over previous
"""Distributed Trainium2 Bass kernel for nn_Attention_25460566131147.

Multi-head attention (B=4, TQ=T=2048, E=2048, H=16, D=128) with gather-based
RoPE and key masking, sharded over 8 NeuronCores: data-parallel over batch
(4 groups) x tensor-parallel over heads (2-way: Wq/Wk/Wv column shards).

Structure (v4):
  - keys are sorted unmasked-first on the host (softmax is permutation-
    invariant over keys) and truncated to KA=1792: dropped keys are all
    masked (zero attention weight) so the result is exact. Remaining masked
    keys land in the last NMASK chunks and are zeroed after exp via a
    per-partition mask multiply; every other chunk needs no mask, letting
    exp run as wide bias-free activations (ACT costs (N+352)/1.2 ns).
  - the Q projection is FUSED into the attention phase: attention runs
    q-block-outer / head-inner, and each sweep first projects+RoPEs its own
    512-wide q-block for all heads. Attention is ScalarE(exp)-bound while
    projections are TensorE-bound, so fusing fills each engine's bubbles.
  - softmax normalization is per (head, q-block) via a two-stage pipelined
    flush (denominator ones-matmul + reciprocal one block later, broadcast
    multiply + ship one block after that) so the in-order TensorE queue
    never waits on the VectorE chain.
  - normalized yt q-slices are AllGathered within the pair in 8 pieces
    (head-pair x q-half) as they complete; the out-projection contracts the
    gathered blocks with a host-permuted Wo (rank-independent) computing
    this core's E-half, ordered so the final AllGathers are hidden behind
    matmuls on already-arrived data.
  - scores are computed transposed (S^T[k,q]) so the exp'd tile feeds P@V
    directly; the softmax denominator comes from a pairwise add + running
    accumulator on VectorE plus a single ones-column matmul (GpSimd is
    avoided: each collective trigger blocks its queue ~20us).
  - a PE warm-up matmul chain covers the initial DMA wait so the HAM clock
    gate reaches 2.4GHz before real work; phase weights/tables are
    prefetched a phase ahead (SBUF pools are strict LIFO per side).
"""

import os
import sys

if "JAX_PLATFORMS" in os.environ and os.environ["JAX_PLATFORMS"] == "axon":
    os.environ["JAX_PLATFORMS"] = "axon,cpu"
sys.path.insert(0, "/opt/trn_rl_repo")

import numpy as np
import ml_dtypes

BF16NP = ml_dtypes.bfloat16

B, TQ, T, E, H, D = 4, 2048, 2048, 2048, 16, 128
BLOCK, THETA = 4096, 10000.0
N_CORES = 8
P = 128

KA = 14 * P               # 1792 active keys kept per batch
NMASK = 2                 # trailing chunks that receive the mask multiply

FULL_CFG = dict(TQ=TQ, T=KA, E=E, HL=8, D=D, NCORES=N_CORES)


def _cs(total, w):
    """Column splits: list of (start, width)."""
    return [(i, min(w, total - i)) for i in range(0, total, w)]


def build_nc(cfg=None):
    """Build and return the (uncompiled) Bacc graph for one SPMD core."""
    import concourse.mybir as mybir
    import concourse.tile as tile
    from concourse import bacc
    from contextlib import ExitStack

    c = dict(FULL_CFG)
    if cfg:
        c.update(cfg)
    cTQ, cT, cE, HL, cD, NCORES = (
        c["TQ"], c["T"], c["E"], c["HL"], c["D"], c["NCORES"],
    )
    assert cD == P
    F = HL * cD              # local feature width (heads shard)
    EC = cE // P             # contraction chunks for projections
    TC = cT // P             # active key chunks (14)
    NQ = 512                 # q-block width
    HQ = cTQ // 2            # q-half width (AllGather granule)
    EH = cE // 2             # out-feature half owned by this core
    BF = mybir.dt.bfloat16
    F32 = mybir.dt.float32
    FR = mybir.dt.float32r
    SCALE = 1.0 / float(np.sqrt(cD))
    groups = [[2 * i, 2 * i + 1] for i in range(NCORES // 2)]

    nc = bacc.Bacc("TRN2", target_bir_lowering=False, debug=False,
                   num_devices=NCORES)

    xt_d = nc.declare_dram_parameter("xt", [cE, cTQ], BF, isOutput=False)
    xat_d = nc.declare_dram_parameter("xat", [cE, cT], BF, isOutput=False)
    wq_d = nc.declare_dram_parameter("wq", [cE, F], BF, isOutput=False)
    wk_d = nc.declare_dram_parameter("wk", [cE, F], BF, isOutput=False)
    wv_d = nc.declare_dram_parameter("wv", [cE, F], BF, isOutput=False)
    # host-permuted Wo rows (AllGather block order), this core's E-col half
    wo_d = nc.declare_dram_parameter("wo", [2 * F, EH], BF, isOutput=False)
    cosq_d = nc.declare_dram_parameter("cosq", [P, cTQ], BF, isOutput=False)
    sinq_d = nc.declare_dram_parameter("sinq", [P, cTQ], BF, isOutput=False)
    cosk_d = nc.declare_dram_parameter("cosk", [P, cT], BF, isOutput=False)
    sink_d = nc.declare_dram_parameter("sink", [P, cT], BF, isOutput=False)
    mb_d = nc.declare_dram_parameter("mask01", [P, TC], F32, isOutput=False)
    out_d = nc.declare_dram_parameter("out", [EH, cTQ], BF, isOutput=True)

    # yt exchange: 8 buffers = (head-pair block) x (q half)
    NBLK = HL // 2
    agin = [nc.dram_tensor(f"agin{j}", [2 * P, HQ], BF) for j in range(8)]
    agout = [nc.dram_tensor(f"agout{j}", [4 * P, HQ], BF) for j in range(8)]

    with tile.TileContext(nc) as tc, ExitStack() as ex:
        consts = ex.enter_context(tc.tile_pool(name="consts", bufs=1, side="right"))
        ones_bf = consts.tile([P, 1], BF, tag="ones_bf", name="ones_bf")
        nc.vector.memset(ones_bf[:], 1.0)
        mb_sb = consts.tile([P, TC], F32, tag="mask01", name="mask01")
        nc.sync.dma_start(mb_sb[:], mb_d[:])
        ones_fr = consts.tile([1, P], F32, tag="ones_fr", name="ones_fr")
        nc.vector.memset(ones_fr[:], 1.0)

        vp = ex.enter_context(tc.tile_pool(name="v", bufs=1, side="right"))
        es_kqv = ExitStack()  # kt pool: closed before phase D (SBUF reuse)

        # left-side pool stack, opened in reverse close order (LIFO):
        es_proj = ExitStack()   # V/K projection psum: [V .. K]
        warmp = es_proj.enter_context(
            tc.tile_pool(name="warm", bufs=1, space="PSUM"))
        psproj = es_proj.enter_context(
            tc.tile_pool(name="psproj", bufs=2, space="PSUM"))
        es_tabq = ExitStack()   # Q weights+tables: [pre-V .. end]
        wqp = es_tabq.enter_context(tc.tile_pool(name="wq", bufs=1))
        tabq = es_tabq.enter_context(tc.tile_pool(name="tabq", bufs=1))
        es_xt = ExitStack()     # x^T quarters, double-buffered: [pre-V .. end]
        xtp = es_xt.enter_context(tc.tile_pool(name="xt", bufs=2))
        es_tabk = ExitStack()   # K weights+tables: [pre-V .. K]
        wkp = es_tabk.enter_context(tc.tile_pool(name="wk", bufs=1))
        tabk = es_tabk.enter_context(tc.tile_pool(name="tabk", bufs=1))
        es_xak = ExitStack()    # xall^T quarters for K: [pre-V .. K]
        xakp = es_xak.enter_context(tc.tile_pool(name="xak", bufs=1))

        SEG = min(512, cT)   # projection column-segment width

        # ============ phase V: V = xall @ Wv, [t-part, n-free] ===========
        assert F <= 1024
        v_sb = [vp.tile([P, F], BF, tag=f"v{t}", name=f"v{t}")
                for t in range(TC)]
        with tc.tile_pool(name="wv", bufs=1) as wvp, \
                tc.tile_pool(name="xav", bufs=1) as xavp:
            wv_sb = []
            for e in range(EC):
                t_ = wvp.tile([P, F], BF, tag=f"wv{e}", name=f"wv{e}")
                wv_sb.append(t_)
            # critical first tiles first, then the rest
            seg0_xa = []
            for e in range(EC):
                t_ = xavp.tile([P, SEG], BF, tag=f"xav{e}", name=f"xav{e}")
                nc.sync.dma_start(t_[:], xat_d[e * P:(e + 1) * P, 0:SEG])
                seg0_xa.append(t_)
            nc.sync.dma_start(wv_sb[0][:], wv_d[0:P, :])
            for e in range(1, EC):
                nc.sync.dma_start(wv_sb[e][:], wv_d[e * P:(e + 1) * P, :])
            # PE warm-up chain: matmuls with no input deps keep the HAM
            # activity monitor busy during the initial DMA wait so the
            # first real matmuls run at 2.4GHz instead of 1.2GHz
            dumw = wvp.tile([P, 512], BF, tag="dumw", name="dumw")
            nc.vector.memset(dumw[:], 0.0)
            wps = warmp.tile([1, 512], F32, tag="wps", name="wps")
            for _ in range(48):
                nc.tensor.matmul(wps[0:1, :], ones_bf[:, 0:1], dumw[:],
                                 start=True, stop=True)
            # prefetch K-phase tables+weights (used next phase)
            cosk_sb = tabk.tile([P, cT], BF, tag="cosk", name="cosk")
            sink_sb = tabk.tile([P, cT], BF, tag="sink", name="sink")
            nc.sync.dma_start(cosk_sb[:], cosk_d[:])
            nc.sync.dma_start(sink_sb[:], sink_d[:])
            wk_sb = []
            for e in range(EC):
                t_ = wkp.tile([P, F], BF, tag=f"wk{e}", name=f"wk{e}")
                nc.sync.dma_start(t_[:], wk_d[e * P:(e + 1) * P, :])
                wk_sb.append(t_)
            for h0, hw in _cs(cT, SEG):
                if h0 == 0:
                    xa_sb = seg0_xa
                else:
                    xa_sb = []
                    for e in range(EC):
                        t_ = xavp.tile([P, SEG], BF, tag=f"xav{e}", name=f"xav{e}")
                        nc.sync.dma_start(
                            t_[:, 0:hw], xat_d[e * P:(e + 1) * P, h0:h0 + hw])
                        xa_sb.append(t_)
                for tl in range(hw // P):
                    t = (h0 // P) + tl
                    ps = psproj.tile([P, F], F32, tag="projpsv", name="projpsv")
                    for e in range(EC):
                        for ns, nw in _cs(F, 512):
                            nc.tensor.matmul(
                                ps[:, ns:ns + nw],
                                xa_sb[e][:, tl * P:(tl + 1) * P],
                                wv_sb[e][:, ns:ns + nw],
                                start=(e == 0), stop=(e == EC - 1),
                            )
                    nc.vector.tensor_copy(v_sb[t][:], ps[:, 0:F])

        # ============ phase K: K-proj + RoPE =============================
        ktp = es_kqv.enter_context(tc.tile_pool(name="kt", bufs=1, side="right"))
        kt_sb = [ktp.tile([P, cT], BF, tag=f"kt{m}", name=f"kt{m}")
                 for m in range(HL)]
        # 448-wide segments divide the trimmed key length evenly (a 256-wide
        # tail segment would expose LDWEIGHTS behind short streams)
        SEGK = 448 if cT % 448 == 0 else SEG
        with tc.tile_pool(name="rawk", bufs=1) as rawkp, \
                tc.tile_pool(name="tmpk", bufs=2) as tmpkp:
            first = True
            for h0, hw in _cs(cT, SEGK):
                xa_sb = []
                for e in range(EC):
                    t_ = xakp.tile([P, SEG], BF, tag=f"xak{e}", name=f"xak{e}")
                    nc.sync.dma_start(
                        t_[:, 0:hw], xat_d[e * P:(e + 1) * P, h0:h0 + hw])
                    xa_sb.append(t_)
                if first:
                    # prefetch Q weights+tables behind seg-0 loads
                    first = False
                    cosq_sb = tabq.tile([P, cTQ], BF, tag="cosq", name="cosq")
                    sinq_sb = tabq.tile([P, cTQ], BF, tag="sinq", name="sinq")
                    nc.sync.dma_start(cosq_sb[:], cosq_d[:])
                    nc.sync.dma_start(sinq_sb[:], sinq_d[:])
                    wq_sb = []
                    for e in range(EC):
                        t_ = wqp.tile([P, F], BF, tag=f"wq{e}", name=f"wq{e}")
                        nc.sync.dma_start(t_[:], wq_d[e * P:(e + 1) * P, :])
                        wq_sb.append(t_)
                for m in range(HL):
                    raw = rawkp.tile([P, hw], BF, tag="rawk", name="rawk")
                    swp = rawkp.tile([P, hw], BF, tag="swpk", name="swpk")
                    ps = psproj.tile([P, SEG], F32, tag="projps", name="projps")
                    for e in range(EC):
                        nc.tensor.matmul(
                            ps[:, 0:hw],
                            wk_sb[e][:, m * P:(m + 1) * P],
                            xa_sb[e][:, 0:hw],
                            start=(e == 0), stop=(e == EC - 1),
                        )
                    nc.scalar.copy(raw[:], ps[:, 0:hw])
                    half = P // 2
                    nc.sync.dma_start(swp[0:half, :], raw[half:P, :])
                    nc.sync.dma_start(swp[half:P, :], raw[0:half, :])
                    t1 = tmpkp.tile([P, hw], BF, tag="rope_t1", name="rope_t1")
                    t2 = tmpkp.tile([P, hw], BF, tag="rope_t2", name="rope_t2")
                    nc.vector.tensor_mul(t1[:], raw[:],
                                         cosk_sb[:, h0:h0 + hw])
                    nc.vector.tensor_mul(t2[:], swp[:],
                                         sink_sb[:, h0:h0 + hw])
                    nc.vector.tensor_add(kt_sb[m][:, h0:h0 + hw], t1[:], t2[:])
            # prefetch the first x^T segment for the fused Q/attention phase
            xt0_sb = []
            for e in range(EC):
                t_ = xtp.tile([P, NQ], BF, tag=f"xt{e}", name=f"xt{e}")
                nc.sync.dma_start(t_[:], xt_d[e * P:(e + 1) * P, 0:NQ])
                xt0_sb.append(t_)
        es_xak.close()
        es_tabk.close()
        es_proj.close()

        # ====== phase C: fused Q-projection + attention, q-block sweeps ===
        es_wo = ExitStack()     # out-proj weights (first half; rest in D)
        wop = es_wo.enter_context(tc.tile_pool(name="wo", bufs=1))
        es_att = ExitStack()
        qtqp = es_att.enter_context(tc.tile_pool(name="qtq", bufs=2))
        rawqp = es_att.enter_context(tc.tile_pool(name="rawq", bufs=1))
        tmpqp = es_att.enter_context(tc.tile_pool(name="tmpq", bufs=1))
        ptp = es_att.enter_context(tc.tile_pool(name="pt", bufs=4))
        pt2p = es_att.enter_context(tc.tile_pool(name="pt2", bufs=3))
        accp = es_att.enter_context(tc.tile_pool(name="acc", bufs=2))
        ytqp = es_att.enter_context(tc.tile_pool(name="ytq", bufs=6))
        dstp = es_att.enter_context(tc.tile_pool(name="dst", bufs=4))
        dbp = es_att.enter_context(tc.tile_pool(name="dbc", bufs=2))
        pss = es_att.enter_context(tc.tile_pool(name="pss", bufs=2, space="PSUM"))
        psy = es_att.enter_context(tc.tile_pool(name="psy", bufs=2, space="PSUM"))
        psq = es_att.enter_context(tc.tile_pool(name="psq", bufs=2, space="PSUM"))

        wo_sb = [None] * (2 * F // P)
        GW = 2                    # score chunks per exp group (PSUM-limited)
        GR = [(i, min(i + GW, TC)) for i in range(0, TC, GW)]
        NG = len(GR)              # 7 slots per block

        def stage_a(p):
            # denominator ones-matmul (PSUM slice from the sps rotation) +
            # psum evacuations + reciprocal
            dpst = pss.tile([P, GW * NQ], F32, tag="sps", name="dps")
            nc.tensor.matmul(
                dpst[0:1, 0:NQ], ones_bf[:, 0:1], p["acc"][:],
                start=True, stop=True,
            )
            ytq = ytqp.tile([P, NQ], BF, tag="ytq", name="ytq")
            nc.vector.tensor_copy(ytq[:], p["yps"][:])
            dst = dstp.tile([1, NQ], F32, tag="dst", name="dst")
            nc.vector.tensor_copy(dst[0:1, :], dpst[0:1, 0:NQ])
            nc.vector.reciprocal(dst[0:1, :], dst[0:1, :])
            p["ytq"], p["dst"] = ytq, dst

        def stage_b(p):
            # broadcast 1/den across partitions on GpSimd (off the TensorE
            # critical path), normalize, ship to the pair-exchange buffer
            dbc = dbp.tile([P, NQ], F32, tag="dbc", name="dbc")
            nc.gpsimd.partition_broadcast(dbc[:], p["dst"][0:1, :], channels=P)
            nc.vector.tensor_mul(p["ytq"][:], p["ytq"][:], dbc[:])
            m, qs = p["m"], p["qs"]
            blk, ml = divmod(m, 2)
            h, co = divmod(qs, HQ)
            j = 2 * blk + h
            nc.sync.dma_start(agin[j][ml * P:(ml + 1) * P, co:co + NQ],
                              p["ytq"][:])
            if ml == 1 and co + NQ == HQ:
                nc.gpsimd.collective_compute(
                    "AllGather", mybir.AluOpType.bypass,
                    replica_groups=groups,
                    ins=[agin[j][:]], outs=[agout[j][:]],
                )

        def emit_exp(st, g):
            # exp + mask for score group g (pt consumed by PV 2 slots later)
            c0, c1 = GR[g]
            w = (c1 - c0) * NQ
            pt = ptp.tile([P, GW * NQ], BF, tag="pt", name="pt")
            st["pt"][g] = pt
            nc.scalar.activation(
                pt[:, 0:w], st["sps"][g][:, 0:w],
                mybir.ActivationFunctionType.Exp, scale=SCALE,
            )
            for kc in range(max(c0, TC - NMASK), c1):
                j = kc - c0
                nc.vector.tensor_scalar_mul(
                    pt[:, j * NQ:(j + 1) * NQ],
                    pt[:, j * NQ:(j + 1) * NQ],
                    mb_sb[:, kc:kc + 1],
                )

        def emit_pv(st, g):
            # P@V + denominator pair/chain adds for score group g
            c0, c1 = GR[g]
            pt = st["pt"][g]
            m = st["m"]
            for kc in range(c0, c1):
                j = kc - c0
                nc.tensor.matmul(
                    st["yps"][:, :],
                    v_sb[kc][:, m * P:(m + 1) * P],
                    pt[:, j * NQ:(j + 1) * NQ],
                    start=(kc == 0), stop=(kc == TC - 1),
                )
            for j0 in range(0, c1 - c0, 2):
                pt2 = pt2p.tile([P, NQ], BF, tag="pt2", name="pt2")
                nc.vector.tensor_add(pt2[:], pt[:, j0 * NQ:(j0 + 1) * NQ],
                                     pt[:, (j0 + 1) * NQ:(j0 + 2) * NQ])
                if st["acc"] is None:
                    st["acc"] = pt2
                else:
                    nacc = accp.tile([P, NQ], BF, tag="acc", name="acc")
                    nc.vector.tensor_add(nacc[:], st["acc"][:], pt2[:])
                    st["acc"] = nacc

        qsl = _cs(cTQ, NQ)
        # proj e-chunk per slot: 16 contraction steps over NG slots
        base, extra = divmod(EC, NG)
        PCH = []
        e0 = 0
        for sgi in range(NG):
            n = base + (1 if sgi < extra else 0)
            PCH.append((e0, e0 + n))
            e0 += n

        def qproj_rope(m, qs0, ps, dst_qt):
            # RoPE epilogue once the projection accumulation is complete
            raw = rawqp.tile([P, NQ], BF, tag="rawq", name="rawq")
            swp = rawqp.tile([P, NQ], BF, tag="swpq", name="swpq")
            nc.vector.tensor_copy(raw[:], ps[:, 0:NQ])
            half = P // 2
            nc.sync.dma_start(swp[0:half, :], raw[half:P, :])
            nc.sync.dma_start(swp[half:P, :], raw[0:half, :])
            t1 = tmpqp.tile([P, NQ], BF, tag="rope_t1", name="rope_t1")
            t2 = tmpqp.tile([P, NQ], BF, tag="rope_t2", name="rope_t2")
            nc.vector.tensor_mul(t1[:], raw[:], cosq_sb[:, qs0:qs0 + NQ])
            nc.vector.tensor_mul(t2[:], swp[:], sinq_sb[:, qs0:qs0 + NQ])
            nc.vector.tensor_add(dst_qt[:], t1[:], t2[:])

        def load_xt(si):
            l = []
            for e in range(EC):
                t_ = xtp.tile([P, NQ], BF, tag=f"xt{e}", name=f"xt{e}")
                nc.sync.dma_start(
                    t_[:], xt_d[e * P:(e + 1) * P, qsl[si][0]:qsl[si][0] + NQ])
                l.append(t_)
            return l

        # prologue: project sweep 0 for all heads
        cur_qt = []
        for m in range(HL):
            qtq = qtqp.tile([P, NQ], BF, tag=f"qtq{m}", name=f"qtq{m}")
            ps = psq.tile([P, NQ], F32, tag="projq", name="projq")
            for e in range(EC):
                nc.tensor.matmul(
                    ps[:, 0:NQ],
                    wq_sb[e][:, m * P:(m + 1) * P],
                    xt0_sb[e][:, 0:NQ],
                    start=(e == 0), stop=(e == EC - 1),
                )
            qproj_rope(m, 0, ps, qtq)
            cur_qt.append(qtq)
        xt_next = load_xt(1)

        pend = []                 # blocks awaiting stage A (last) / B (first)
        prev_st = None            # block with exp(g6)/PV(g5,g6) outstanding
        for qsi, (qs, qw) in enumerate(qsl):
            assert qw == NQ
            xt_after = load_xt(qsi + 2) if qsi + 2 < len(qsl) else None
            nxt_qt = []
            for m in range(HL):
                do_proj = qsi + 1 < len(qsl)
                if do_proj:
                    qtq = qtqp.tile([P, NQ], BF, tag=f"qtq{m}", name=f"qtq{m}")
                    pps = psq.tile([P, NQ], F32, tag="projq", name="projq")
                    nxt_qt.append(qtq)
                if qsi == 0 and 1 <= m <= 2:
                    # first half of the out-proj weights (rest loads in D)
                    for f in range(4 * (m - 1), 4 * m):
                        t_ = wop.tile([P, EH], BF, tag=f"wo{f}", name=f"wo{f}")
                        nc.sync.dma_start(t_[:], wo_d[f * P:(f + 1) * P, :])
                        wo_sb[f] = t_
                st = {
                    "yps": psy.tile([P, NQ], F32, tag="yps", name="yps"),
                    "sps": [None] * NG,
                    "pt": [None] * NG,
                    "acc": None,
                    "m": m,
                    "qs": qs,
                }
                # modulo-scheduled slots: [exp(g-1)] [S(g)] [proj chunk]
                # [PV(g-2)]; PV lags its exp by 2 slots so the in-order
                # TensorE queue never waits on the ACT engine
                for g, (c0, c1) in enumerate(GR):
                    if g == 0:
                        if prev_st is not None:
                            emit_exp(prev_st, NG - 1)
                    else:
                        emit_exp(st, g - 1)
                    sps = pss.tile([P, GW * NQ], F32, tag="sps", name="sps")
                    st["sps"][g] = sps
                    for kc in range(c0, c1):
                        j = kc - c0
                        nc.tensor.matmul(
                            sps[:, j * NQ:(j + 1) * NQ],
                            kt_sb[m][:, kc * P:(kc + 1) * P],
                            cur_qt[m][:],
                            start=True, stop=True,
                        )
                    if do_proj:
                        pe0, pe1 = PCH[g]
                        for e in range(pe0, pe1):
                            nc.tensor.matmul(
                                pps[:, 0:NQ],
                                wq_sb[e][:, m * P:(m + 1) * P],
                                xt_next[e][:, 0:NQ],
                                start=(e == 0), stop=(e == EC - 1),
                            )
                    if g == 0:
                        if prev_st is not None:
                            emit_pv(prev_st, NG - 2)
                        if len(pend) >= 3:
                            stage_b(pend.pop(0))
                    elif g == 1:
                        if prev_st is not None:
                            emit_pv(prev_st, NG - 1)
                        if pend:
                            stage_a(pend[-1])
                    else:
                        emit_pv(st, g - 2)
                if do_proj:
                    qproj_rope(m, qsl[qsi + 1][0], pps, qtq)
                prev_st = st
                pend.append(st)
            cur_qt = nxt_qt if nxt_qt else cur_qt
            xt_next = xt_after
        # epilogue: drain the deferred tail and the stage pipeline
        emit_exp(prev_st, NG - 1)
        emit_pv(prev_st, NG - 2)
        emit_pv(prev_st, NG - 1)
        stage_a(pend[-1])
        for p in pend:
            stage_b(p)
        es_att.close()
        es_kqv.close()

        # ================= phase D: out-projection =======================
        # out^T[EH, q] = Wo'^T @ ya. For the second q-half the f contraction
        # is emitted in two sweeps over all 8 PSUM banks — f 0..11 first —
        # so ~20us of matmuls hide the final AllGathers and their loads.
        NT = EH // P
        NF = 2 * HL
        with tc.tile_pool(name="ya", bufs=1) as yap, \
                tc.tile_pool(name="oev", bufs=4) as oevp, \
                tc.tile_pool(name="pso", bufs=8, space="PSUM") as pso:
            for f in range(8, 2 * F // P):
                # second half of the out-proj weights (SBUF was too tight
                # to stage them during attention)
                t_ = yap.tile([P, EH], BF, tag=f"wo{f}", name=f"wo{f}")
                nc.sync.dma_start(t_[:], wo_d[f * P:(f + 1) * P, :])
                wo_sb[f] = t_
            ya_sb = []
            for f in range(NF):
                t_ = yap.tile([P, cTQ], BF, tag=f"ya{f}", name=f"ya{f}")
                ya_sb.append(t_)
            for h in range(2):
                for f in range(NF):
                    blk, r = divmod(f, 4)
                    nc.sync.dma_start(
                        ya_sb[f][:, h * HQ:(h + 1) * HQ],
                        agout[2 * blk + h][r * P:(r + 1) * P, :])
            for ms, mw in _cs(cTQ, 512):
                sweeps = [(0, NF)] if ms < HQ else [(0, 12), (12, NF)]
                opss = []
                for si, (f0, f1) in enumerate(sweeps):
                    for n in range(NT):
                        if si == 0:
                            ops = pso.tile([P, 512], F32, tag="ops", name="ops")
                            opss.append(ops)
                        else:
                            ops = opss[n]
                        for f in range(f0, f1):
                            nc.tensor.matmul(
                                ops[:, 0:mw],
                                wo_sb[f][:, n * P:(n + 1) * P],
                                ya_sb[f][:, ms:ms + mw],
                                start=(f == 0), stop=(f == NF - 1),
                            )
                        if si == len(sweeps) - 1:
                            oev = oevp.tile([P, 512], BF, tag="oev", name="oev")
                            if n % 2 == 0:
                                nc.scalar.copy(oev[:, 0:mw], ops[:, 0:mw])
                            else:
                                nc.vector.tensor_copy(oev[:, 0:mw],
                                                      ops[:, 0:mw])
                            nc.sync.dma_start(
                                out_d[n * P:(n + 1) * P, ms:ms + mw],
                                oev[:, 0:mw])
        es_wo.close()
        es_xt.close()
        es_tabq.close()

    return nc


# ---------------------------------------------------------------------------
# host side
# ---------------------------------------------------------------------------

def _rope_tables():
    inv_freq = 1.0 / (THETA ** (np.arange(0, D, 2, dtype=np.float32) / D))
    t = np.arange(BLOCK, dtype=np.float32)
    freqs = np.einsum("i,j->ij", t, inv_freq).astype(np.float32)
    emb = np.concatenate([freqs, freqs], axis=-1)
    return np.cos(emb).astype(np.float32), np.sin(emb).astype(np.float32)


_NC_CACHE = {}


def _get_compiled():
    if "nc" not in _NC_CACHE:
        nc = build_nc()
        nc.compile()
        _NC_CACHE["nc"] = nc
    return _NC_CACHE["nc"]


def _bf(a):
    return np.ascontiguousarray(a).astype(BF16NP)


def prepare_in_maps(x, xall, posx, posxall, mask, Wq, Wk, Wv, Wo):
    x = np.asarray(x, dtype=np.float32)
    xall = np.asarray(xall, dtype=np.float32)
    posx = np.asarray(posx)
    posxall = np.asarray(posxall)
    mask = np.asarray(mask)
    Wq = np.asarray(Wq, dtype=np.float32)
    Wk = np.asarray(Wk, dtype=np.float32)
    Wv = np.asarray(Wv, dtype=np.float32)
    Wo = np.asarray(Wo, dtype=np.float32)

    cos_t, sin_t = _rope_tables()
    sign = np.ones((1, D), np.float32)
    sign[0, : D // 2] = -1.0

    F = (H * D) // 2  # 1024: per-core head-shard width
    FB = 2 * D        # 256: AllGather block (2 heads)
    # AllGather block order: [A blk_i, B blk_i] for i in 0..3, where A/B are
    # the pair's rank-0/rank-1 feature halves of Wo's rows
    Wo_perm = np.concatenate(
        [w for i in range(4)
         for w in (Wo[i * FB:(i + 1) * FB], Wo[F + i * FB:F + (i + 1) * FB])],
        axis=0)

    in_maps = []
    for c in range(N_CORES):
        b, hg = c // 2, c % 2
        sl = slice(hg * F, (hg + 1) * F)
        # sort keys unmasked-first (stable) and keep the first KA: every
        # dropped key is masked (zero attention weight) so this is exact;
        # remaining masked keys land in the last NMASK chunks
        order = np.argsort(mask[b], kind="stable")[:KA]
        act = int((~mask[b]).sum())
        assert act <= KA, f"batch {b}: {act} active keys > {KA}"
        assert KA - act <= NMASK * P, \
            f"batch {b}: masked keys spill out of the last {NMASK} chunks"
        xall_p = xall[b][order]
        posk_p = posxall[b][order]
        mask_p = mask[b][order]
        cosq = _bf(cos_t[posx[b]].T)                    # [128, TQ]
        sinq = _bf((sin_t[posx[b]] * sign).T)
        cosk = _bf(cos_t[posk_p].T)                     # [128, KA]
        sink = _bf((sin_t[posk_p] * sign).T)
        m01 = np.where(mask_p, np.float32(0.0), np.float32(1.0))
        m01 = np.ascontiguousarray(m01.reshape(KA // P, P).T)  # [128, TCA]
        in_maps.append({
            "xt": _bf(x[b].T),
            "xat": _bf(xall_p.T),
            "wq": _bf(Wq[:, sl]),
            "wk": _bf(Wk[:, sl]),
            "wv": _bf(Wv[:, sl]),
            "wo": _bf(Wo_perm[:, hg * (E // 2):(hg + 1) * (E // 2)]),
            "cosq": cosq, "sinq": sinq, "cosk": cosk, "sink": sink,
            "mask01": m01.astype(np.float32),
        })
    return in_maps


def assemble_out(results):
    # core (b, hg) computed out^T for E columns [hg*E/2, (hg+1)*E/2)
    EH = E // 2
    out = np.empty((B, TQ, E), np.float32)
    for b in range(B):
        for hg in range(2):
            half = results[2 * b + hg]["out"].astype(np.float32)
            out[b][:, hg * EH:(hg + 1) * EH] = half.T
    return out


def kernel(x, xall, posx, posxall, mask, Wq, Wk, Wv, Wo):
    from concourse.bass_utils import run_bass_kernel_spmd

    in_maps = prepare_in_maps(x, xall, posx, posxall, mask, Wq, Wk, Wv, Wo)
    nc = _get_compiled()
    res = run_bass_kernel_spmd(nc, in_maps, list(range(N_CORES)), trace=False)
    return assemble_out(res.results)


# revision 45
# speedup vs baseline: 1.1910x; 1.0452x over previous
"""Distributed Trainium2 Bass kernel for nn_Attention_25460566131147.

Multi-head attention (B=4, TQ=T=2048, E=2048, H=16, D=128) with gather-based
RoPE and key masking, sharded over 8 NeuronCores: data-parallel over batch
(4 groups) x tensor-parallel over heads (2-way: Wq/Wk/Wv column shards).

Structure (v4):
  - keys are sorted unmasked-first on the host (softmax is permutation-
    invariant over keys) and truncated to KA=1792: dropped keys are all
    masked (zero attention weight) so the result is exact. Remaining masked
    keys land in the last NMASK chunks and are zeroed after exp via a
    per-partition mask multiply; every other chunk needs no mask, letting
    exp run as wide bias-free activations (ACT costs (N+352)/1.2 ns).
  - the Q projection is FUSED into the attention phase: attention runs
    q-block-outer / head-inner, and each sweep first projects+RoPEs its own
    512-wide q-block for all heads. Attention is ScalarE(exp)-bound while
    projections are TensorE-bound, so fusing fills each engine's bubbles.
  - softmax normalization is per (head, q-block) via a two-stage pipelined
    flush (denominator ones-matmul + reciprocal one block later, broadcast
    multiply + ship one block after that) so the in-order TensorE queue
    never waits on the VectorE chain.
  - normalized yt q-slices are AllGathered within the pair in 8 pieces
    (head-pair x q-half) as they complete; the out-projection contracts the
    gathered blocks with a host-permuted Wo (rank-independent) computing
    this core's E-half, ordered so the final AllGathers are hidden behind
    matmuls on already-arrived data.
  - scores are computed transposed (S^T[k,q]) so the exp'd tile feeds P@V
    directly; the softmax denominator comes from a pairwise add + running
    accumulator on VectorE plus a single ones-column matmul (GpSimd is
    avoided: each collective trigger blocks its queue ~20us).
  - a PE warm-up matmul chain covers the initial DMA wait so the HAM clock
    gate reaches 2.4GHz before real work; phase weights/tables are
    prefetched a phase ahead (SBUF pools are strict LIFO per side).
"""

import os
import sys

if "JAX_PLATFORMS" in os.environ and os.environ["JAX_PLATFORMS"] == "axon":
    os.environ["JAX_PLATFORMS"] = "axon,cpu"
sys.path.insert(0, "/opt/trn_rl_repo")

import numpy as np
import ml_dtypes

BF16NP = ml_dtypes.bfloat16

B, TQ, T, E, H, D = 4, 2048, 2048, 2048, 16, 128
BLOCK, THETA = 4096, 10000.0
N_CORES = 8
P = 128

KA = 14 * P               # 1792 active keys kept per batch
NMASK = 2                 # trailing chunks that receive the mask multiply

FULL_CFG = dict(TQ=TQ, T=KA, E=E, HL=8, D=D, NCORES=N_CORES)


def _cs(total, w):
    """Column splits: list of (start, width)."""
    return [(i, min(w, total - i)) for i in range(0, total, w)]


def build_nc(cfg=None):
    """Build and return the (uncompiled) Bacc graph for one SPMD core."""
    import concourse.mybir as mybir
    import concourse.tile as tile
    from concourse import bacc
    from contextlib import ExitStack

    c = dict(FULL_CFG)
    if cfg:
        c.update(cfg)
    cTQ, cT, cE, HL, cD, NCORES = (
        c["TQ"], c["T"], c["E"], c["HL"], c["D"], c["NCORES"],
    )
    assert cD == P
    F = HL * cD              # local feature width (heads shard)
    EC = cE // P             # contraction chunks for projections
    TC = cT // P             # active key chunks (14)
    NQ = 512                 # q-block width
    HQ = cTQ // 2            # q-half width (AllGather granule)
    EH = cE // 2             # out-feature half owned by this core
    BF = mybir.dt.bfloat16
    F32 = mybir.dt.float32
    FR = mybir.dt.float32r
    SCALE = 1.0 / float(np.sqrt(cD))
    groups = [[2 * i, 2 * i + 1] for i in range(NCORES // 2)]

    nc = bacc.Bacc("TRN2", target_bir_lowering=False, debug=False,
                   num_devices=NCORES)

    xt_d = nc.declare_dram_parameter("xt", [cE, cTQ], BF, isOutput=False)
    xat_d = nc.declare_dram_parameter("xat", [cE, cT], BF, isOutput=False)
    wq_d = nc.declare_dram_parameter("wq", [cE, F], BF, isOutput=False)
    wk_d = nc.declare_dram_parameter("wk", [cE, F], BF, isOutput=False)
    wv_d = nc.declare_dram_parameter("wv", [cE, F], BF, isOutput=False)
    # host-permuted Wo rows (AllGather block order), this core's E-col half
    wo_d = nc.declare_dram_parameter("wo", [2 * F, EH], BF, isOutput=False)
    cosq_d = nc.declare_dram_parameter("cosq", [P, cTQ], BF, isOutput=False)
    sinq_d = nc.declare_dram_parameter("sinq", [P, cTQ], BF, isOutput=False)
    cosk_d = nc.declare_dram_parameter("cosk", [P, cT], BF, isOutput=False)
    sink_d = nc.declare_dram_parameter("sink", [P, cT], BF, isOutput=False)
    mb_d = nc.declare_dram_parameter("mask01", [P, TC], F32, isOutput=False)
    out_d = nc.declare_dram_parameter("out", [EH, cTQ], BF, isOutput=True)

    # yt exchange: 8 buffers = (head-pair block) x (q half)
    NBLK = HL // 2
    agin = [nc.dram_tensor(f"agin{j}", [2 * P, HQ], BF) for j in range(8)]
    agout = [nc.dram_tensor(f"agout{j}", [4 * P, HQ], BF) for j in range(8)]

    with tile.TileContext(nc) as tc, ExitStack() as ex:
        consts = ex.enter_context(tc.tile_pool(name="consts", bufs=1, side="right"))
        ones_bf = consts.tile([P, 1], BF, tag="ones_bf", name="ones_bf")
        nc.vector.memset(ones_bf[:], 1.0)
        mb_sb = consts.tile([P, TC], F32, tag="mask01", name="mask01")
        nc.sync.dma_start(mb_sb[:], mb_d[:])
        ones_fr = consts.tile([1, P], F32, tag="ones_fr", name="ones_fr")
        nc.vector.memset(ones_fr[:], 1.0)

        vp = ex.enter_context(tc.tile_pool(name="v", bufs=1, side="right"))
        es_kqv = ExitStack()  # kt pool: closed before phase D (SBUF reuse)

        # left-side pool stack, opened in reverse close order (LIFO):
        es_proj = ExitStack()   # V/K projection psum: [V .. K]
        warmp = es_proj.enter_context(
            tc.tile_pool(name="warm", bufs=1, space="PSUM"))
        psproj = es_proj.enter_context(
            tc.tile_pool(name="psproj", bufs=2, space="PSUM"))
        es_tabq = ExitStack()   # Q weights+tables: [pre-V .. end]
        wqp = es_tabq.enter_context(tc.tile_pool(name="wq", bufs=1))
        tabq = es_tabq.enter_context(tc.tile_pool(name="tabq", bufs=1))
        es_xt = ExitStack()     # x^T quarters, double-buffered: [pre-V .. end]
        xtp = es_xt.enter_context(tc.tile_pool(name="xt", bufs=2))
        es_tabk = ExitStack()   # K weights+tables: [pre-V .. K]
        wkp = es_tabk.enter_context(tc.tile_pool(name="wk", bufs=1))
        tabk = es_tabk.enter_context(tc.tile_pool(name="tabk", bufs=1))
        es_xak = ExitStack()    # xall^T quarters for K: [pre-V .. K]
        xakp = es_xak.enter_context(tc.tile_pool(name="xak", bufs=1))

        SEG = min(512, cT)   # projection column-segment width

        # ============ phase V: V = xall @ Wv, [t-part, n-free] ===========
        assert F <= 1024
        v_sb = [vp.tile([P, F], BF, tag=f"v{t}", name=f"v{t}")
                for t in range(TC)]
        with tc.tile_pool(name="wv", bufs=1) as wvp, \
                tc.tile_pool(name="xav", bufs=1) as xavp:
            wv_sb = []
            for e in range(EC):
                t_ = wvp.tile([P, F], BF, tag=f"wv{e}", name=f"wv{e}")
                wv_sb.append(t_)
            # critical first tiles first, then the rest
            seg0_xa = []
            for e in range(EC):
                t_ = xavp.tile([P, SEG], BF, tag=f"xav{e}", name=f"xav{e}")
                nc.sync.dma_start(t_[:], xat_d[e * P:(e + 1) * P, 0:SEG])
                seg0_xa.append(t_)
            nc.sync.dma_start(wv_sb[0][:], wv_d[0:P, :])
            for e in range(1, EC):
                nc.sync.dma_start(wv_sb[e][:], wv_d[e * P:(e + 1) * P, :])
            # PE warm-up chain: matmuls with no input deps keep the HAM
            # activity monitor busy during the initial DMA wait so the
            # first real matmuls run at 2.4GHz instead of 1.2GHz
            dumw = wvp.tile([P, 512], BF, tag="dumw", name="dumw")
            nc.vector.memset(dumw[:], 0.0)
            wps = warmp.tile([1, 512], F32, tag="wps", name="wps")
            for _ in range(48):
                nc.tensor.matmul(wps[0:1, :], ones_bf[:, 0:1], dumw[:],
                                 start=True, stop=True)
            # prefetch K-phase tables+weights (used next phase)
            cosk_sb = tabk.tile([P, cT], BF, tag="cosk", name="cosk")
            sink_sb = tabk.tile([P, cT], BF, tag="sink", name="sink")
            nc.sync.dma_start(cosk_sb[:], cosk_d[:])
            nc.sync.dma_start(sink_sb[:], sink_d[:])
            wk_sb = []
            for e in range(EC):
                t_ = wkp.tile([P, F], BF, tag=f"wk{e}", name=f"wk{e}")
                nc.sync.dma_start(t_[:], wk_d[e * P:(e + 1) * P, :])
                wk_sb.append(t_)
            for h0, hw in _cs(cT, SEG):
                if h0 == 0:
                    xa_sb = seg0_xa
                else:
                    xa_sb = []
                    for e in range(EC):
                        t_ = xavp.tile([P, SEG], BF, tag=f"xav{e}", name=f"xav{e}")
                        nc.sync.dma_start(
                            t_[:, 0:hw], xat_d[e * P:(e + 1) * P, h0:h0 + hw])
                        xa_sb.append(t_)
                for tl in range(hw // P):
                    t = (h0 // P) + tl
                    ps = psproj.tile([P, F], F32, tag="projpsv", name="projpsv")
                    for e in range(EC):
                        for ns, nw in _cs(F, 512):
                            nc.tensor.matmul(
                                ps[:, ns:ns + nw],
                                xa_sb[e][:, tl * P:(tl + 1) * P],
                                wv_sb[e][:, ns:ns + nw],
                                start=(e == 0), stop=(e == EC - 1),
                            )
                    nc.vector.tensor_copy(v_sb[t][:], ps[:, 0:F])
        # zero masked keys' V rows (keys live on partitions): P@V then needs
        # no masked attention weights, and only the softmax denominator
        # needs the mask (folded into its add tree)
        for kc in range(TC - NMASK, TC):
            nc.vector.tensor_scalar_mul(v_sb[kc][:], v_sb[kc][:],
                                        mb_sb[:, kc:kc + 1])

        # ============ phase K: K-proj + RoPE =============================
        ktp = es_kqv.enter_context(tc.tile_pool(name="kt", bufs=1, side="right"))
        kt_sb = [ktp.tile([P, cT], BF, tag=f"kt{m}", name=f"kt{m}")
                 for m in range(HL)]
        # 448-wide segments divide the trimmed key length evenly (a 256-wide
        # tail segment would expose LDWEIGHTS behind short streams)
        SEGK = 448 if cT % 448 == 0 else SEG
        with tc.tile_pool(name="rawk", bufs=1) as rawkp, \
                tc.tile_pool(name="tmpk", bufs=2) as tmpkp:
            first = True
            for h0, hw in _cs(cT, SEGK):
                xa_sb = []
                for e in range(EC):
                    t_ = xakp.tile([P, SEG], BF, tag=f"xak{e}", name=f"xak{e}")
                    nc.sync.dma_start(
                        t_[:, 0:hw], xat_d[e * P:(e + 1) * P, h0:h0 + hw])
                    xa_sb.append(t_)
                if first:
                    # prefetch Q weights+tables behind seg-0 loads
                    first = False
                    cosq_sb = tabq.tile([P, cTQ], BF, tag="cosq", name="cosq")
                    sinq_sb = tabq.tile([P, cTQ], BF, tag="sinq", name="sinq")
                    nc.sync.dma_start(cosq_sb[:], cosq_d[:])
                    nc.sync.dma_start(sinq_sb[:], sinq_d[:])
                    wq_sb = []
                    for e in range(EC):
                        t_ = wqp.tile([P, F], BF, tag=f"wq{e}", name=f"wq{e}")
                        nc.sync.dma_start(t_[:], wq_d[e * P:(e + 1) * P, :])
                        wq_sb.append(t_)
                for m in range(HL):
                    raw = rawkp.tile([P, hw], BF, tag="rawk", name="rawk")
                    swp = rawkp.tile([P, hw], BF, tag="swpk", name="swpk")
                    ps = psproj.tile([P, SEG], F32, tag="projps", name="projps")
                    for e in range(EC):
                        nc.tensor.matmul(
                            ps[:, 0:hw],
                            wk_sb[e][:, m * P:(m + 1) * P],
                            xa_sb[e][:, 0:hw],
                            start=(e == 0), stop=(e == EC - 1),
                        )
                    nc.scalar.copy(raw[:], ps[:, 0:hw])
                    half = P // 2
                    nc.sync.dma_start(swp[0:half, :], raw[half:P, :])
                    nc.sync.dma_start(swp[half:P, :], raw[0:half, :])
                    t1 = tmpkp.tile([P, hw], BF, tag="rope_t1", name="rope_t1")
                    t2 = tmpkp.tile([P, hw], BF, tag="rope_t2", name="rope_t2")
                    nc.vector.tensor_mul(t1[:], raw[:],
                                         cosk_sb[:, h0:h0 + hw])
                    nc.vector.tensor_mul(t2[:], swp[:],
                                         sink_sb[:, h0:h0 + hw])
                    nc.vector.tensor_add(kt_sb[m][:, h0:h0 + hw], t1[:], t2[:])
            # prefetch the first x^T segment for the fused Q/attention phase
            xt0_sb = []
            for e in range(EC):
                t_ = xtp.tile([P, NQ], BF, tag=f"xt{e}", name=f"xt{e}")
                nc.sync.dma_start(t_[:], xt_d[e * P:(e + 1) * P, 0:NQ])
                xt0_sb.append(t_)
        es_xak.close()
        es_tabk.close()
        es_proj.close()

        # ====== phase C: fused Q-projection + attention, q-block sweeps ===
        es_wo = ExitStack()     # out-proj weights (first half; rest in D)
        wop = es_wo.enter_context(tc.tile_pool(name="wo", bufs=1))
        es_att = ExitStack()
        qtqp = es_att.enter_context(tc.tile_pool(name="qtq", bufs=2))
        rawqp = es_att.enter_context(tc.tile_pool(name="rawq", bufs=1))
        tmpqp = es_att.enter_context(tc.tile_pool(name="tmpq", bufs=1))
        ptp = es_att.enter_context(tc.tile_pool(name="pt", bufs=4))
        trp = es_att.enter_context(tc.tile_pool(name="tr", bufs=4))
        wp = es_att.enter_context(tc.tile_pool(name="w", bufs=2))
        ytqp = es_att.enter_context(tc.tile_pool(name="ytq", bufs=5))
        dstp = es_att.enter_context(tc.tile_pool(name="dst", bufs=3))
        dbp = es_att.enter_context(tc.tile_pool(name="dbc", bufs=2))
        pss = es_att.enter_context(tc.tile_pool(name="pss", bufs=2, space="PSUM"))
        psy = es_att.enter_context(tc.tile_pool(name="psy", bufs=2, space="PSUM"))
        psq = es_att.enter_context(tc.tile_pool(name="psq", bufs=2, space="PSUM"))

        wo_sb = [None] * (2 * F // P)
        GW = 2                    # score chunks per exp group (PSUM-limited)
        GR = [(i, min(i + GW, TC)) for i in range(0, TC, GW)]
        NG = len(GR)              # 7 slots per block

        def stage_a(p):
            # denominator ones-matmul over the masked-folded tree halves
            # (PSUM slice from the sps rotation) + evacuations + reciprocal
            dpst = pss.tile([P, GW * NQ], F32, tag="sps", name="dps")
            nc.tensor.matmul(
                dpst[0:1, 0:NQ], ones_bf[:, 0:1], p["w0"][:],
                start=True, stop=False,
            )
            nc.tensor.matmul(
                dpst[0:1, 0:NQ], ones_bf[:, 0:1], p["w1"][:],
                start=False, stop=True,
            )
            ytq = ytqp.tile([P, NQ], BF, tag="ytq", name="ytq")
            nc.scalar.copy(ytq[:], p["yps"][:])
            dst = dstp.tile([1, NQ], F32, tag="dst", name="dst")
            nc.vector.tensor_copy(dst[0:1, :], dpst[0:1, 0:NQ])
            nc.vector.reciprocal(dst[0:1, :], dst[0:1, :])
            p["ytq"], p["dst"] = ytq, dst

        def stage_b_data(p):
            # broadcast 1/den across partitions on GpSimd (off the TensorE
            # critical path), normalize, ship to the pair-exchange buffer
            dbc = dbp.tile([P, NQ], F32, tag="dbc", name="dbc")
            nc.gpsimd.partition_broadcast(dbc[:], p["dst"][0:1, :], channels=P)
            nc.vector.tensor_mul(p["ytq"][:], p["ytq"][:], dbc[:])
            m, qs = p["m"], p["qs"]
            blk, ml = divmod(m, 2)
            h, co = divmod(qs, HQ)
            j = 2 * blk + h
            nc.sync.dma_start(agin[j][ml * P:(ml + 1) * P, co:co + NQ],
                              p["ytq"][:])
            return j if (ml == 1 and co + NQ == HQ) else None

        def fire_ag(j):
            nc.gpsimd.collective_compute(
                "AllGather", mybir.AluOpType.bypass,
                replica_groups=groups,
                ins=[agin[j][:]], outs=[agout[j][:]],
            )

        def stage_b(p):
            j = stage_b_data(p)
            if j is not None:
                fire_ag(j)

        def emit_exp(st, g):
            # exp for score group g (pt consumed by PV 2 slots later)
            c0, c1 = GR[g]
            w = (c1 - c0) * NQ
            pt = ptp.tile([P, GW * NQ], BF, tag="pt", name="pt")
            st["pt"][g] = pt
            nc.scalar.activation(
                pt[:, 0:w], st["sps"][g][:, 0:w],
                mybir.ActivationFunctionType.Exp, scale=SCALE,
            )

        def emit_pv(st, g):
            # P@V for score group g, plus this block's share of the wide
            # denominator add-tree (few wide VectorE ops instead of many
            # narrow ones: DVE fixed cost dominates at [128,512])
            c0, c1 = GR[g]
            pt = st["pt"][g]
            m = st["m"]
            for kc in range(c0, c1):
                j = kc - c0
                nc.tensor.matmul(
                    st["yps"][:, :],
                    v_sb[kc][:, m * P:(m + 1) * P],
                    pt[:, j * NQ:(j + 1) * NQ],
                    start=(kc == 0), stop=(kc == TC - 1),
                )
            pts = st["pt"]
            if g == 1:
                st["t01"] = trp.tile([P, GW * NQ], BF, tag="tr", name="tr")
                nc.vector.tensor_add(st["t01"][:], pts[0][:], pts[1][:])
            elif g == 3:
                t23 = trp.tile([P, GW * NQ], BF, tag="tr", name="tr")
                nc.vector.tensor_add(t23[:], pts[2][:], pts[3][:])
                st["u"] = trp.tile([P, GW * NQ], BF, tag="tr", name="tr")
                nc.vector.tensor_add(st["u"][:], st["t01"][:], t23[:])
            elif g == 5:
                t45 = trp.tile([P, GW * NQ], BF, tag="tr", name="tr")
                nc.vector.tensor_add(t45[:], pts[4][:], pts[5][:])
                st["v"] = trp.tile([P, GW * NQ], BF, tag="tr", name="tr")
                nc.vector.tensor_add(st["v"][:], st["u"][:], t45[:])
            elif g == 6:
                # fold the (masked) last group into the tree halves:
                # w = (pt6 * mask01) + v, one fused op per half
                for hh in range(2):
                    wt = wp.tile([P, NQ], BF, tag=f"w{hh}", name=f"w{hh}")
                    st[f"w{hh}"] = wt
                    nc.vector.scalar_tensor_tensor(
                        wt[:],
                        pts[6][:, hh * NQ:(hh + 1) * NQ],
                        mb_sb[:, TC - NMASK + hh:TC - NMASK + hh + 1],
                        st["v"][:, hh * NQ:(hh + 1) * NQ],
                        mybir.AluOpType.mult,
                        mybir.AluOpType.add,
                    )

        qsl = _cs(cTQ, NQ)
        # proj e-chunk per slot: 16 contraction steps over NG slots
        base, extra = divmod(EC, NG)
        PCH = []
        e0 = 0
        for sgi in range(NG):
            n = base + (1 if sgi < extra else 0)
            PCH.append((e0, e0 + n))
            e0 += n

        def qproj_rope(m, qs0, ps, dst_qt):
            # RoPE epilogue once the projection accumulation is complete
            raw = rawqp.tile([P, NQ], BF, tag="rawq", name="rawq")
            swp = rawqp.tile([P, NQ], BF, tag="swpq", name="swpq")
            nc.vector.tensor_copy(raw[:], ps[:, 0:NQ])
            half = P // 2
            nc.sync.dma_start(swp[0:half, :], raw[half:P, :])
            nc.sync.dma_start(swp[half:P, :], raw[0:half, :])
            t1 = tmpqp.tile([P, NQ], BF, tag="rope_t1", name="rope_t1")
            t2 = tmpqp.tile([P, NQ], BF, tag="rope_t2", name="rope_t2")
            nc.vector.tensor_mul(t1[:], raw[:], cosq_sb[:, qs0:qs0 + NQ])
            nc.vector.tensor_mul(t2[:], swp[:], sinq_sb[:, qs0:qs0 + NQ])
            nc.vector.tensor_add(dst_qt[:], t1[:], t2[:])

        def load_xt(si):
            l = []
            for e in range(EC):
                t_ = xtp.tile([P, NQ], BF, tag=f"xt{e}", name=f"xt{e}")
                nc.sync.dma_start(
                    t_[:], xt_d[e * P:(e + 1) * P, qsl[si][0]:qsl[si][0] + NQ])
                l.append(t_)
            return l

        # prologue: project sweep 0 for all heads
        cur_qt = []
        for m in range(HL):
            qtq = qtqp.tile([P, NQ], BF, tag=f"qtq{m}", name=f"qtq{m}")
            ps = psq.tile([P, NQ], F32, tag="projq", name="projq")
            for e in range(EC):
                nc.tensor.matmul(
                    ps[:, 0:NQ],
                    wq_sb[e][:, m * P:(m + 1) * P],
                    xt0_sb[e][:, 0:NQ],
                    start=(e == 0), stop=(e == EC - 1),
                )
            qproj_rope(m, 0, ps, qtq)
            cur_qt.append(qtq)
        xt_next = load_xt(1)

        pend = []                 # blocks awaiting stage A (last) / B (first)
        prev_st = None            # block with exp(g6)/PV(g5,g6) outstanding
        for qsi, (qs, qw) in enumerate(qsl):
            assert qw == NQ
            xt_after = load_xt(qsi + 2) if qsi + 2 < len(qsl) else None
            nxt_qt = []
            for m in range(HL):
                do_proj = qsi + 1 < len(qsl)
                if do_proj:
                    qtq = qtqp.tile([P, NQ], BF, tag=f"qtq{m}", name=f"qtq{m}")
                    pps = psq.tile([P, NQ], F32, tag="projq", name="projq")
                    nxt_qt.append(qtq)
                if qsi == 0 and 1 <= m <= 2:
                    # first half of the out-proj weights (rest loads in D)
                    for f in range(4 * (m - 1), 4 * m):
                        t_ = wop.tile([P, EH], BF, tag=f"wo{f}", name=f"wo{f}")
                        nc.sync.dma_start(t_[:], wo_d[f * P:(f + 1) * P, :])
                        wo_sb[f] = t_
                st = {
                    "yps": psy.tile([P, NQ], F32, tag="yps", name="yps"),
                    "sps": [None] * NG,
                    "pt": [None] * NG,
                    "m": m,
                    "qs": qs,
                }
                # modulo-scheduled slots: [exp(g-1)] [S(g)] [proj chunk]
                # [PV(g-2)]; PV lags its exp by 2 slots so the in-order
                # TensorE queue never waits on the ACT engine
                for g, (c0, c1) in enumerate(GR):
                    if g == 0:
                        if prev_st is not None:
                            emit_exp(prev_st, NG - 1)
                    else:
                        emit_exp(st, g - 1)
                    sps = pss.tile([P, GW * NQ], F32, tag="sps", name="sps")
                    st["sps"][g] = sps
                    for kc in range(c0, c1):
                        j = kc - c0
                        nc.tensor.matmul(
                            sps[:, j * NQ:(j + 1) * NQ],
                            kt_sb[m][:, kc * P:(kc + 1) * P],
                            cur_qt[m][:],
                            start=True, stop=True,
                        )
                    if do_proj:
                        pe0, pe1 = PCH[g]
                        for e in range(pe0, pe1):
                            nc.tensor.matmul(
                                pps[:, 0:NQ],
                                wq_sb[e][:, m * P:(m + 1) * P],
                                xt_next[e][:, 0:NQ],
                                start=(e == 0), stop=(e == EC - 1),
                            )
                    if g == 0:
                        if prev_st is not None:
                            emit_pv(prev_st, NG - 2)
                        if len(pend) >= 2:
                            stage_b(pend.pop(0))
                    elif g == 1:
                        if prev_st is not None:
                            emit_pv(prev_st, NG - 1)
                        if pend:
                            stage_a(pend[-1])
                    else:
                        emit_pv(st, g - 2)
                if do_proj:
                    qproj_rope(m, qsl[qsi + 1][0], pps, qtq)
                prev_st = st
                pend.append(st)
            cur_qt = nxt_qt if nxt_qt else cur_qt
            xt_next = xt_after
        # epilogue: drain the deferred tail and the stage pipeline; data
        # ops first, AllGather triggers last (a trigger blocks the GpSimd
        # queue until the collective completes, which would serialize the
        # remaining broadcasts behind it)
        emit_exp(prev_st, NG - 1)
        emit_pv(prev_st, NG - 2)
        emit_pv(prev_st, NG - 1)
        stage_a(pend[-1])
        ags = [stage_b_data(p) for p in pend]
        for j in ags:
            if j is not None:
                fire_ag(j)
        es_att.close()
        es_kqv.close()

        # ================= phase D: out-projection =======================
        # out^T[EH, q] = Wo'^T @ ya. For the second q-half the f contraction
        # is emitted in two sweeps over all 8 PSUM banks — f 0..11 first —
        # so ~20us of matmuls hide the final AllGathers and their loads.
        NT = EH // P
        NF = 2 * HL
        with tc.tile_pool(name="ya", bufs=1) as yap, \
                tc.tile_pool(name="oev", bufs=4) as oevp, \
                tc.tile_pool(name="pso", bufs=8, space="PSUM") as pso:
            ya_sb = []
            for f in range(NF):
                t_ = yap.tile([P, cTQ], BF, tag=f"ya{f}", name=f"ya{f}")
                ya_sb.append(t_)

            def ya_dma(f, h):
                blk, r = divmod(f, 4)
                nc.sync.dma_start(
                    ya_sb[f][:, h * HQ:(h + 1) * HQ],
                    agout[2 * blk + h][r * P:(r + 1) * P, :])

            # load order mirrors consumption: ya f0-7 h0 (first sweep),
            # wo second half + ya f8-15 h0 (second sweep), then the h1s
            for f in range(8):
                ya_dma(f, 0)
            for f in range(8, 2 * F // P):
                t_ = yap.tile([P, EH], BF, tag=f"wo{f}", name=f"wo{f}")
                nc.sync.dma_start(t_[:], wo_d[f * P:(f + 1) * P, :])
                wo_sb[f] = t_
            for f in range(8, NF):
                ya_dma(f, 0)
            for f in range(NF):
                ya_dma(f, 1)
            for ms, mw in _cs(cTQ, 512):
                if ms == 0:
                    # f 0..7 are in SBUF since attention; sweeping them
                    # first hides the wo/ya loads emitted just above
                    sweeps = [(0, 8), (8, NF)]
                elif ms < HQ:
                    sweeps = [(0, NF)]
                else:
                    sweeps = [(0, 12), (12, NF)]
                opss = []
                for si, (f0, f1) in enumerate(sweeps):
                    for n in range(NT):
                        if si == 0:
                            ops = pso.tile([P, 512], F32, tag="ops", name="ops")
                            opss.append(ops)
                        else:
                            ops = opss[n]
                        for f in range(f0, f1):
                            nc.tensor.matmul(
                                ops[:, 0:mw],
                                wo_sb[f][:, n * P:(n + 1) * P],
                                ya_sb[f][:, ms:ms + mw],
                                start=(f == 0), stop=(f == NF - 1),
                            )
                        if si == len(sweeps) - 1:
                            oev = oevp.tile([P, 512], BF, tag="oev", name="oev")
                            if n % 2 == 0:
                                nc.scalar.copy(oev[:, 0:mw], ops[:, 0:mw])
                            else:
                                nc.vector.tensor_copy(oev[:, 0:mw],
                                                      ops[:, 0:mw])
                            nc.sync.dma_start(
                                out_d[n * P:(n + 1) * P, ms:ms + mw],
                                oev[:, 0:mw])
        es_wo.close()
        es_xt.close()
        es_tabq.close()

    return nc


# ---------------------------------------------------------------------------
# host side
# ---------------------------------------------------------------------------

def _rope_tables():
    inv_freq = 1.0 / (THETA ** (np.arange(0, D, 2, dtype=np.float32) / D))
    t = np.arange(BLOCK, dtype=np.float32)
    freqs = np.einsum("i,j->ij", t, inv_freq).astype(np.float32)
    emb = np.concatenate([freqs, freqs], axis=-1)
    return np.cos(emb).astype(np.float32), np.sin(emb).astype(np.float32)


_NC_CACHE = {}


def _get_compiled():
    if "nc" not in _NC_CACHE:
        nc = build_nc()
        nc.compile()
        _NC_CACHE["nc"] = nc
    return _NC_CACHE["nc"]


def _bf(a):
    return np.ascontiguousarray(a).astype(BF16NP)


def prepare_in_maps(x, xall, posx, posxall, mask, Wq, Wk, Wv, Wo):
    x = np.asarray(x, dtype=np.float32)
    xall = np.asarray(xall, dtype=np.float32)
    posx = np.asarray(posx)
    posxall = np.asarray(posxall)
    mask = np.asarray(mask)
    Wq = np.asarray(Wq, dtype=np.float32)
    Wk = np.asarray(Wk, dtype=np.float32)
    Wv = np.asarray(Wv, dtype=np.float32)
    Wo = np.asarray(Wo, dtype=np.float32)

    cos_t, sin_t = _rope_tables()
    sign = np.ones((1, D), np.float32)
    sign[0, : D // 2] = -1.0

    F = (H * D) // 2  # 1024: per-core head-shard width
    FB = 2 * D        # 256: AllGather block (2 heads)
    # AllGather block order: [A blk_i, B blk_i] for i in 0..3, where A/B are
    # the pair's rank-0/rank-1 feature halves of Wo's rows
    Wo_perm = np.concatenate(
        [w for i in range(4)
         for w in (Wo[i * FB:(i + 1) * FB], Wo[F + i * FB:F + (i + 1) * FB])],
        axis=0)

    in_maps = []
    for c in range(N_CORES):
        b, hg = c // 2, c % 2
        sl = slice(hg * F, (hg + 1) * F)
        # sort keys unmasked-first (stable) and keep the first KA: every
        # dropped key is masked (zero attention weight) so this is exact;
        # remaining masked keys land in the last NMASK chunks
        order = np.argsort(mask[b], kind="stable")[:KA]
        act = int((~mask[b]).sum())
        assert act <= KA, f"batch {b}: {act} active keys > {KA}"
        assert KA - act <= NMASK * P, \
            f"batch {b}: masked keys spill out of the last {NMASK} chunks"
        xall_p = xall[b][order]
        posk_p = posxall[b][order]
        mask_p = mask[b][order]
        cosq = _bf(cos_t[posx[b]].T)                    # [128, TQ]
        sinq = _bf((sin_t[posx[b]] * sign).T)
        cosk = _bf(cos_t[posk_p].T)                     # [128, KA]
        sink = _bf((sin_t[posk_p] * sign).T)
        m01 = np.where(mask_p, np.float32(0.0), np.float32(1.0))
        m01 = np.ascontiguousarray(m01.reshape(KA // P, P).T)  # [128, TCA]
        in_maps.append({
            "xt": _bf(x[b].T),
            "xat": _bf(xall_p.T),
            "wq": _bf(Wq[:, sl]),
            "wk": _bf(Wk[:, sl]),
            "wv": _bf(Wv[:, sl]),
            "wo": _bf(Wo_perm[:, hg * (E // 2):(hg + 1) * (E // 2)]),
            "cosq": cosq, "sinq": sinq, "cosk": cosk, "sink": sink,
            "mask01": m01.astype(np.float32),
        })
    return in_maps


def assemble_out(results):
    # core (b, hg) computed out^T for E columns [hg*E/2, (hg+1)*E/2)
    EH = E // 2
    out = np.empty((B, TQ, E), np.float32)
    for b in range(B):
        for hg in range(2):
            half = results[2 * b + hg]["out"].astype(np.float32)
            out[b][:, hg * EH:(hg + 1) * EH] = half.T
    return out


def kernel(x, xall, posx, posxall, mask, Wq, Wk, Wv, Wo):
    from concourse.bass_utils import run_bass_kernel_spmd

    in_maps = prepare_in_maps(x, xall, posx, posxall, mask, Wq, Wk, Wv, Wo)
    nc = _get_compiled()
    res = run_bass_kernel_spmd(nc, in_maps, list(range(N_CORES)), trace=False)
    return assemble_out(res.results)
